# revision 3
# baseline (speedup 1.0000x reference)
"""HGCN 2-layer GNN message passing kernel for 8 Trainium2 NeuronCores.

Math notes (vs the reference):
  - alpha = softmax over a size-1 axis == 1.0 exactly, so the attention
    branch (Wa, ba, leaky_relu, softmax) contributes nothing.
  - msg = x_j * (-|curv|), so each layer is
        out = segment_sum((x @ W + b)[src], dst) * s      with s = -|curv|
    and since matmul distributes over the segment sum:
        out = segment_sum(x[src], dst) @ (W*s) + deg * (b*s)
    i.e. aggregate raw features first, apply the (scaled) linear after.
  - layer1: h = relu(out1); layer2: log_softmax(out2).

Sharding: nodes are range-partitioned across the 8 cores by destination
(6250 nodes each).  Each core processes the edges whose dst lands in its
range.  Edges are sorted by dst on the host; per 128-node dst block the
core gathers x[src] rows with dma_gather (int16 indices force a low/high
table split at 32768) and segment-sums them with one-hot matmuls
accumulated in PSUM.  Gather tables are bf16 (PSUM accumulation stays
f32).  Between layers the per-core h slices are AllGathered so every core
can gather layer-2 messages from the full table.
"""

import os
import sys

import numpy as np

if "/opt/trn_rl_repo" not in sys.path:
    sys.path.insert(0, "/opt/trn_rl_repo")

import concourse.bacc as bacc
import concourse.bass as bass
import concourse.mybir as mybir
import concourse.tile as tile
from concourse.bass_utils import run_bass_kernel_spmd

P = 128
N_CORES = 8
SPLIT = 32768  # int16 index limit for dma_gather
GBLK = 4  # dst blocks per gather superblock


# ---------------------------------------------------------------------------
# host-side edge preprocessing
# ---------------------------------------------------------------------------

def _wrap_idx(raw):
    """[n*128] row indices -> [128, n*8] int16 dma_gather index layout
    (wrapped in 16 partitions, replicated across the 8 gpsimd cores)."""
    n = raw.shape[0]
    w = raw.reshape(n // 16, 16).T.astype(np.int16)  # [16, n//16]
    return np.tile(w, (8, 1))


def _preprocess(edge_index, n_nodes, n_cores):
    """Sort edges (plus self loops) by dst, split per core / per 128-dst
    block / by src<SPLIT, and build the padded per-core index arrays.

    Chunk column order groups blocks into superblocks of GBLK so each
    superblock needs just two dma_gathers (low table / high table):
      [sb0: lows of b0..b3 | highs of b0..b3][sb1: ...]
    """
    src = np.concatenate([edge_index[0], np.arange(n_nodes, dtype=np.int64)])
    dst = np.concatenate([edge_index[1], np.arange(n_nodes, dtype=np.int64)])
    order = np.argsort(dst, kind="stable")
    src_s = src[order].astype(np.int64)
    dst_s = dst[order].astype(np.int64)
    deg = np.bincount(dst, minlength=n_nodes).astype(np.float32)

    npc = n_nodes // n_cores  # nodes per core
    nblk = (npc + P - 1) // P
    lows, highs, dlows, dhighs = {}, {}, {}, {}
    cl = np.zeros((n_cores, nblk), dtype=np.int64)
    ch = np.zeros((n_cores, nblk), dtype=np.int64)
    for c in range(n_cores):
        for b in range(nblk):
            lo = c * npc + b * P
            hi = min(lo + P, (c + 1) * npc)
            e0 = np.searchsorted(dst_s, lo, side="left")
            e1 = np.searchsorted(dst_s, hi, side="left")
            s = src_s[e0:e1]
            dl = (dst_s[e0:e1] - lo).astype(np.float32)
            m = s < SPLIT
            lows[c, b], dlows[c, b] = s[m], dl[m]
            highs[c, b], dhighs[c, b] = s[~m] - SPLIT, dl[~m]
            cl[c, b] = (len(lows[c, b]) + P - 1) // P
            ch[c, b] = (len(highs[c, b]) + P - 1) // P
    CL = np.maximum(cl.max(axis=0), 1)  # shared program: max chunks per block
    CH = np.maximum(ch.max(axis=0), 1)
    totc = int(CL.sum() + CH.sum())

    # chunk-column order: per superblock, lows of its blocks then highs
    nsb = (nblk + GBLK - 1) // GBLK
    col_of = {}  # (b, "lo"/"hi") -> first chunk column
    col = 0
    for g in range(nsb):
        bs = range(g * GBLK, min((g + 1) * GBLK, nblk))
        for b in bs:
            col_of[b, "lo"] = col
            col += int(CL[b])
        for b in bs:
            col_of[b, "hi"] = col
            col += int(CH[b])
    assert col == totc

    idx_arrs, dloc_arrs = [], []
    for c in range(n_cores):
        idx_np = np.zeros((P, 8 * totc), dtype=np.int16)
        dloc_np = np.full((P, totc), float(P), dtype=np.float32)
        for b in range(nblk):
            for key, arrs, darrs, nch in (
                ("lo", lows, dlows, CL[b]),
                ("hi", highs, dhighs, CH[b]),
            ):
                a = arrs[c, b]
                d = darrs[c, b]
                ni = int(nch) * P
                pad_a = np.zeros(ni, dtype=np.int64)
                pad_a[: len(a)] = a
                pad_d = np.full(ni, float(P), dtype=np.float32)
                pad_d[: len(d)] = d
                c0 = col_of[b, key]
                idx_np[:, 8 * c0 : 8 * (c0 + int(nch))] = _wrap_idx(pad_a)
                dloc_np[:, c0 : c0 + int(nch)] = pad_d.reshape(int(nch), P).T
        idx_arrs.append(idx_np)
        dloc_arrs.append(dloc_np.astype(np.float32))
    return CL, CH, col_of, idx_arrs, dloc_arrs, deg


# ---------------------------------------------------------------------------
# device program
# ---------------------------------------------------------------------------

def _build_program(CL, CH, col_of, n_nodes, d_in, d_hid, d_out, n_cores):
    npc = n_nodes // n_cores
    nblk = len(CL)
    nsb = (nblk + GBLK - 1) // GBLK
    totc = int(CL.sum() + CH.sum())
    f32 = mybir.dt.float32
    bf16 = mybir.dt.bfloat16
    # max chunks handled by one superblock gather tile
    sb_blocks = [list(range(g * GBLK, min((g + 1) * GBLK, nblk))) for g in range(nsb)]
    sb_cl = [int(sum(CL[b] for b in bs)) for bs in sb_blocks]
    sb_ch = [int(sum(CH[b] for b in bs)) for bs in sb_blocks]
    sbmax = max(cl + ch for cl, ch in zip(sb_cl, sb_ch))
    cmax_blk = int((CL + CH).max())

    nc = bacc.Bacc(
        "TRN2",
        target_bir_lowering=False,
        debug=False,
        num_devices=n_cores,
        num_swdge_queues=4,
    )
    xb_ap = nc.dram_tensor("xb", [n_nodes, d_in], bf16, kind="ExternalInput").ap()
    w1_ap = nc.dram_tensor("w1", [d_in, d_hid], f32, kind="ExternalInput").ap()
    w2_ap = nc.dram_tensor("w2", [d_hid, d_out], f32, kind="ExternalInput").ap()
    idx_ap = nc.dram_tensor("idx", [P, 8 * totc], mybir.dt.int16, kind="ExternalInput").ap()
    dloc_ap = nc.dram_tensor("dloc", [P, totc], bf16, kind="ExternalInput").ap()
    degb1_ap = nc.dram_tensor("degb1", [npc, d_hid], f32, kind="ExternalInput").ap()
    degb2_ap = nc.dram_tensor("degb2", [npc, d_out], f32, kind="ExternalInput").ap()
    out_ap = nc.dram_tensor("out", [npc, d_out], f32, kind="ExternalOutput").ap()

    gq = [0]  # rotating swdge queue assignment

    def gather(gt, table, idx_sb, col, nch, elem):
        ni = int(nch) * P
        nc.gpsimd.dma_gather(
            out_ap=gt.rearrange("p (c e) -> p c e", e=elem),
            in_ap=table,
            idxs_ap=idx_sb[:, 8 * col : 8 * (col + int(nch))],
            num_idxs=ni,
            num_idxs_reg=ni,
            elem_size=elem,
            single_packet=False,
            queue_num=gq[0] % 4,
        )
        gq[0] += 1

    with tile.TileContext(nc) as tc:
        with (
            tc.tile_pool(name="const", bufs=1) as cp,
            tc.tile_pool(name="g", bufs=2) as gxp,
            tc.tile_pool(name="oh", bufs=2) as ohp,
            tc.tile_pool(name="blk", bufs=3) as bp,
            tc.tile_pool(name="dram", bufs=1, space="DRAM") as dram,
        ):
            w1_sb = cp.tile([d_in, d_hid], f32)
            w2_sb = cp.tile([d_hid, d_out], f32)
            idx_sb = cp.tile([P, 8 * totc], mybir.dt.int16)
            dloc_sb = cp.tile([P, totc], bf16)
            iota_b = cp.tile([P, cmax_blk * P], bf16)
            nc.sync.dma_start(out=w1_sb[:], in_=w1_ap[:])
            nc.sync.dma_start(out=w2_sb[:], in_=w2_ap[:])
            nc.sync.dma_start(out=idx_sb[:], in_=idx_ap[:])
            nc.sync.dma_start(out=dloc_sb[:], in_=dloc_ap[:])
            nc.gpsimd.iota(
                iota_b[:].rearrange("p (c m) -> p c m", m=P),
                pattern=[[0, cmax_blk], [1, P]],
                base=0,
                channel_multiplier=0,
                allow_small_or_imprecise_dtypes=True,
            )

            hslice = dram.tile([npc, d_hid], bf16)
            hfull = dram.tile([n_nodes, d_hid], bf16, addr_space="Shared")

            def onehot(col, nch):
                """one-hot [128 edges, nch chunks * 128 nodes], bf16."""
                n = int(nch)
                oh = ohp.tile([P, cmax_blk * P], bf16, tag="oh")
                nc.vector.tensor_tensor(
                    out=oh[:, : n * P].rearrange("p (c m) -> p c m", m=P),
                    in0=dloc_sb[:, col : col + n, None].to_broadcast([P, n, P]),
                    in1=iota_b[:, : n * P].rearrange("p (c m) -> p c m", m=P),
                    op=mybir.AluOpType.is_equal,
                )
                return oh

            def phase(layer):
                """layer 1: gather x_bf16, agg -> @W1f +degb1, relu -> h slice.
                layer 2: gather h_bf16, agg -> @W2f +degb2, log_softmax -> out."""
                table = xb_ap if layer == 1 else hfull[:]
                d_row = d_in if layer == 1 else d_hid
                for g in range(nsb):
                    bs = sb_blocks[g]
                    gt = gxp.tile([P, sbmax * d_row], bf16, tag="g")
                    c0 = col_of[bs[0], "lo"]
                    gather(
                        gt[:, : sb_cl[g] * d_row],
                        table[:SPLIT, :],
                        idx_sb,
                        c0,
                        sb_cl[g],
                        d_row,
                    )
                    gather(
                        gt[:, sb_cl[g] * d_row : (sb_cl[g] + sb_ch[g]) * d_row],
                        table[SPLIT:, :],
                        idx_sb,
                        c0 + sb_cl[g],
                        sb_ch[g],
                        d_row,
                    )
                    for b in bs:
                        nbsz = min(P, npc - b * P)
                        # aggT[f, node] += sum over the block's chunks
                        aggT = (psA if layer == 1 else psA2).tile(
                            [P, P], f32, space="PSUM", tag="aggT"
                        )
                        chunks = [
                            (col_of[b, "lo"], int(CL[b])),
                            (col_of[b, "hi"], int(CH[b])),
                        ]
                        ctot = int(CL[b] + CH[b])
                        kk = 0
                        for cstart, cn in chunks:
                            oh = onehot(cstart, cn)
                            for k in range(cn):
                                gcol = (cstart - c0) * d_row
                                nc.tensor.matmul(
                                    out=aggT[:d_row, :],
                                    lhsT=gt[:, gcol + k * d_row : gcol + (k + 1) * d_row],
                                    rhs=oh[:, k * P : (k + 1) * P],
                                    start=(kk == 0),
                                    stop=(kk == ctot - 1),
                                )
                                kk += 1
                        aggT_sb = bp.tile([P, P], f32, tag="aggT_sb")
                        nc.vector.tensor_copy(out=aggT_sb[:d_row, :], in_=aggT[:d_row, :])
                        d_o = d_hid if layer == 1 else d_out
                        w_sb = w1_sb if layer == 1 else w2_sb
                        degb = degb1_ap if layer == 1 else degb2_ap
                        o_ps = (psH if layer == 1 else psO).tile(
                            [P, d_o], f32, space="PSUM", tag="o"
                        )
                        nc.tensor.matmul(
                            out=o_ps[:], lhsT=aggT_sb[:d_row, :], rhs=w_sb[:],
                            start=True, stop=True,
                        )
                        db = bp.tile([P, d_o], f32, tag="db")
                        nc.sync.dma_start(
                            out=db[:nbsz, :], in_=degb[b * P : b * P + nbsz, :]
                        )
                        t_sb = bp.tile([P, d_o], f32, tag="t_sb")
                        nc.vector.tensor_add(
                            out=t_sb[:nbsz, :], in0=o_ps[:nbsz, :], in1=db[:nbsz, :]
                        )
                        if layer == 1:
                            hb = bp.tile([P, d_hid], bf16, tag="hb")
                            nc.scalar.activation(
                                out=hb[:nbsz, :], in_=t_sb[:nbsz, :],
                                func=mybir.ActivationFunctionType.Relu,
                            )
                            nc.sync.dma_start(
                                out=hslice[b * P : b * P + nbsz, :], in_=hb[:nbsz, :]
                            )
                        else:
                            mx = bp.tile([P, 1], f32, tag="mx")
                            nc.vector.reduce_max(
                                out=mx[:nbsz, :], in_=t_sb[:nbsz, :],
                                axis=mybir.AxisListType.X,
                            )
                            tm = bp.tile([P, d_o], f32, tag="tm")
                            nc.vector.tensor_scalar_sub(
                                out=tm[:nbsz, :], in0=t_sb[:nbsz, :], scalar1=mx[:nbsz, :]
                            )
                            ex = bp.tile([P, d_o], f32, tag="ex")
                            nc.scalar.activation(
                                out=ex[:nbsz, :], in_=tm[:nbsz, :],
                                func=mybir.ActivationFunctionType.Exp,
                            )
                            sm = bp.tile([P, 1], f32, tag="sm")
                            nc.vector.reduce_sum(
                                out=sm[:nbsz, :], in_=ex[:nbsz, :],
                                axis=mybir.AxisListType.X,
                            )
                            ls = bp.tile([P, 1], f32, tag="ls")
                            nc.scalar.activation(
                                out=ls[:nbsz, :], in_=sm[:nbsz, :],
                                func=mybir.ActivationFunctionType.Ln,
                            )
                            res = bp.tile([P, d_o], f32, tag="res")
                            nc.vector.tensor_scalar_sub(
                                out=res[:nbsz, :], in0=tm[:nbsz, :], scalar1=ls[:nbsz, :]
                            )
                            nc.sync.dma_start(
                                out=out_ap[b * P : b * P + nbsz, :], in_=res[:nbsz, :]
                            )

            with (
                tc.tile_pool(name="psA", bufs=2, space="PSUM") as psA,
                tc.tile_pool(name="psH", bufs=2, space="PSUM") as psH,
            ):
                phase(1)

            nc.gpsimd.collective_compute(
                "AllGather",
                mybir.AluOpType.bypass,
                replica_groups=[list(range(n_cores))],
                ins=[hslice[:].opt()],
                outs=[hfull[:].opt()],
            )

            with (
                tc.tile_pool(name="psA2", bufs=2, space="PSUM") as psA2,
                tc.tile_pool(name="psO", bufs=2, space="PSUM") as psO,
            ):
                phase(2)

    nc.compile()
    return nc


_PROGRAM_CACHE = {}


def _run(x, edge_index, W1f, b1f, W2f, b2f, n_cores=N_CORES):
    n_nodes, d_in = x.shape
    d_hid = W1f.shape[1]
    d_out = W2f.shape[1]
    npc = n_nodes // n_cores

    CL, CH, col_of, idx_arrs, dloc_arrs, deg = _preprocess(edge_index, n_nodes, n_cores)

    key = (n_nodes, d_in, d_hid, d_out, n_cores, tuple(CL), tuple(CH))
    if key not in _PROGRAM_CACHE:
        _PROGRAM_CACHE[key] = _build_program(
            CL, CH, col_of, n_nodes, d_in, d_hid, d_out, n_cores
        )
    nc = _PROGRAM_CACHE[key]

    import ml_dtypes

    xb = np.ascontiguousarray(x.astype(ml_dtypes.bfloat16))
    in_maps = []
    for c in range(n_cores):
        deg_c = deg[c * npc : (c + 1) * npc]
        in_maps.append(
            {
                "xb": xb,
                "w1": np.ascontiguousarray(W1f),
                "w2": np.ascontiguousarray(W2f),
                "idx": idx_arrs[c],
                "dloc": np.ascontiguousarray(dloc_arrs[c].astype(ml_dtypes.bfloat16)),
                "degb1": np.ascontiguousarray(np.outer(deg_c, b1f).astype(np.float32)),
                "degb2": np.ascontiguousarray(np.outer(deg_c, b2f).astype(np.float32)),
            }
        )
    res = run_bass_kernel_spmd(
        nc,
        in_maps,
        core_ids=list(range(n_cores)),
        trace=bool(os.environ.get("KERNEL_TRACE")),
    )
    out = np.concatenate([res.results[c]["out"] for c in range(n_cores)], axis=0)
    return out, res


def kernel(x, edge_index, W1, b1, Wa1, ba1, curv1, W2, b2, Wa2, ba2, curv2):
    x = np.asarray(x, dtype=np.float32)
    edge_index = np.asarray(edge_index).astype(np.int64)
    s1 = -abs(float(np.asarray(curv1).reshape(-1)[0]))
    s2 = -abs(float(np.asarray(curv2).reshape(-1)[0]))
    W1f = np.asarray(W1, dtype=np.float32) * s1
    b1f = np.asarray(b1, dtype=np.float32) * s1
    W2f = np.asarray(W2, dtype=np.float32) * s2
    b2f = np.asarray(b2, dtype=np.float32) * s2
    out, _ = _run(x, edge_index, W1f, b1f, W2f, b2f)
    return out


# revision 7
# speedup vs baseline: 1.0307x; 1.0307x over previous
"""HGCN 2-layer GNN message passing kernel for 8 Trainium2 NeuronCores.

Math notes (vs the reference):
  - alpha = softmax over a size-1 axis == 1.0 exactly, so the attention
    branch (Wa, ba, leaky_relu, softmax) contributes nothing.
  - msg = x_j * (-|curv|), so each layer is
        out = segment_sum((x @ W + b)[src], dst) * s      with s = -|curv|
    and since matmul distributes over the segment sum:
        out = segment_sum(x[src], dst) @ (W*s) + deg * (b*s)
    i.e. aggregate raw features first, apply the (scaled) linear after.
  - layer1: h = relu(out1); layer2: log_softmax(out2).

Sharding: nodes are range-partitioned across the 8 cores by destination
(6250 nodes each).  Each core processes the edges whose dst lands in its
range.  Edges are sorted by dst on the host; per 128-node dst block the
core gathers x[src] rows with dma_gather (int16 indices force a low/high
table split at 32768) and segment-sums them with one-hot matmuls
accumulated in PSUM.  Gather tables are bf16 (PSUM accumulation stays
f32).  Between layers the per-core h slices are AllGathered so every core
can gather layer-2 messages from the full table.
"""

import os
import sys

import numpy as np

if "/opt/trn_rl_repo" not in sys.path:
    sys.path.insert(0, "/opt/trn_rl_repo")

import concourse.bacc as bacc
import concourse.bass as bass
import concourse.mybir as mybir
import concourse.tile as tile
from concourse.bass_utils import run_bass_kernel_spmd

P = 128
N_CORES = 8
SPLIT = 32768  # int16 index limit for dma_gather
GBLK = 4  # dst blocks per gather superblock


# ---------------------------------------------------------------------------
# host-side edge preprocessing
# ---------------------------------------------------------------------------

def _wrap_idx(raw):
    """[n*128] row indices -> [128, n*8] int16 dma_gather index layout
    (wrapped in 16 partitions, replicated across the 8 gpsimd cores)."""
    n = raw.shape[0]
    w = raw.reshape(n // 16, 16).T.astype(np.int16)  # [16, n//16]
    return np.tile(w, (8, 1))


def _preprocess(edge_index, n_nodes, n_cores):
    """Sort edges (plus self loops) by dst, split per core / per 128-dst
    block / by src<SPLIT, and build the padded per-core index arrays.

    Chunk column order groups blocks into superblocks of GBLK so each
    superblock needs just two dma_gathers (low table / high table):
      [sb0: lows of b0..b3 | highs of b0..b3][sb1: ...]
    """
    src = np.concatenate([edge_index[0], np.arange(n_nodes, dtype=np.int64)])
    dst = np.concatenate([edge_index[1], np.arange(n_nodes, dtype=np.int64)])
    order = np.argsort(dst, kind="stable")
    src_s = src[order].astype(np.int64)
    dst_s = dst[order].astype(np.int64)
    deg = np.bincount(dst, minlength=n_nodes).astype(np.float32)

    npc = n_nodes // n_cores  # nodes per core
    nblk = (npc + P - 1) // P
    lows, highs, dlows, dhighs = {}, {}, {}, {}
    cl = np.zeros((n_cores, nblk), dtype=np.int64)
    ch = np.zeros((n_cores, nblk), dtype=np.int64)
    for c in range(n_cores):
        for b in range(nblk):
            lo = c * npc + b * P
            hi = min(lo + P, (c + 1) * npc)
            e0 = np.searchsorted(dst_s, lo, side="left")
            e1 = np.searchsorted(dst_s, hi, side="left")
            s = src_s[e0:e1]
            dl = (dst_s[e0:e1] - lo).astype(np.float32)
            m = s < SPLIT
            lows[c, b], dlows[c, b] = s[m], dl[m]
            highs[c, b], dhighs[c, b] = s[~m] - SPLIT, dl[~m]
            cl[c, b] = (len(lows[c, b]) + P - 1) // P
            ch[c, b] = (len(highs[c, b]) + P - 1) // P
    CL = np.maximum(cl.max(axis=0), 1)  # shared program: max chunks per block
    CH = np.maximum(ch.max(axis=0), 1)
    totc = int(CL.sum() + CH.sum())

    # chunk-column order: per superblock, lows of its blocks then highs
    nsb = (nblk + GBLK - 1) // GBLK
    col_of = {}  # (b, "lo"/"hi") -> first chunk column
    col = 0
    for g in range(nsb):
        bs = range(g * GBLK, min((g + 1) * GBLK, nblk))
        for b in bs:
            col_of[b, "lo"] = col
            col += int(CL[b])
        for b in bs:
            col_of[b, "hi"] = col
            col += int(CH[b])
    assert col == totc

    idx_arrs, dloc_arrs = [], []
    for c in range(n_cores):
        idx_np = np.zeros((P, 8 * totc), dtype=np.int16)
        dloc_np = np.full((P, totc), float(P), dtype=np.float32)
        for b in range(nblk):
            for key, arrs, darrs, nch in (
                ("lo", lows, dlows, CL[b]),
                ("hi", highs, dhighs, CH[b]),
            ):
                a = arrs[c, b]
                d = darrs[c, b]
                ni = int(nch) * P
                pad_a = np.zeros(ni, dtype=np.int64)
                pad_a[: len(a)] = a
                pad_d = np.full(ni, float(P), dtype=np.float32)
                pad_d[: len(d)] = d
                c0 = col_of[b, key]
                idx_np[:, 8 * c0 : 8 * (c0 + int(nch))] = _wrap_idx(pad_a)
                dloc_np[:, c0 : c0 + int(nch)] = pad_d.reshape(int(nch), P).T
        idx_arrs.append(idx_np)
        dloc_arrs.append(dloc_np.astype(np.float32))
    return CL, CH, col_of, idx_arrs, dloc_arrs, deg


# ---------------------------------------------------------------------------
# device program
# ---------------------------------------------------------------------------

def _build_program(CL, CH, col_of, n_nodes, d_in, d_hid, d_out, n_cores):
    npc = n_nodes // n_cores
    nblk = len(CL)
    nsb = (nblk + GBLK - 1) // GBLK
    totc = int(CL.sum() + CH.sum())
    f32 = mybir.dt.float32
    bf16 = mybir.dt.bfloat16
    # max chunks handled by one superblock gather tile
    sb_blocks = [list(range(g * GBLK, min((g + 1) * GBLK, nblk))) for g in range(nsb)]
    sb_cl = [int(sum(CL[b] for b in bs)) for bs in sb_blocks]
    sb_ch = [int(sum(CH[b] for b in bs)) for bs in sb_blocks]
    sbmax = max(cl + ch for cl, ch in zip(sb_cl, sb_ch))
    cmax_blk = int((CL + CH).max())

    nc = bacc.Bacc(
        "TRN2",
        target_bir_lowering=False,
        debug=False,
        num_devices=n_cores,
        num_swdge_queues=4,
    )
    xb_ap = nc.dram_tensor("xb", [n_nodes, d_in], bf16, kind="ExternalInput").ap()
    w1_ap = nc.dram_tensor("w1", [d_in, d_hid], f32, kind="ExternalInput").ap()
    w2_ap = nc.dram_tensor("w2", [d_hid, d_out], f32, kind="ExternalInput").ap()
    idx_ap = nc.dram_tensor("idx", [P, 8 * totc], mybir.dt.int16, kind="ExternalInput").ap()
    dloc_ap = nc.dram_tensor("dloc", [P, totc], bf16, kind="ExternalInput").ap()
    degb1_ap = nc.dram_tensor("degb1", [npc, d_hid], f32, kind="ExternalInput").ap()
    degb2_ap = nc.dram_tensor("degb2", [npc, d_out], f32, kind="ExternalInput").ap()
    out_ap = nc.dram_tensor("out", [npc, d_out], f32, kind="ExternalOutput").ap()

    gq = [0]  # rotating swdge queue assignment

    def gather(gt, table, idx_sb, col, nch, elem):
        ni = int(nch) * P
        nc.gpsimd.dma_gather(
            out_ap=gt.rearrange("p (c e) -> p c e", e=elem),
            in_ap=table,
            idxs_ap=idx_sb[:, 8 * col : 8 * (col + int(nch))],
            num_idxs=ni,
            num_idxs_reg=ni,
            elem_size=elem,
            single_packet=False,
            queue_num=gq[0] % 4,
        )
        gq[0] += 1

    with tile.TileContext(nc) as tc:
        with (
            tc.tile_pool(name="const", bufs=1) as cp,
            tc.tile_pool(name="g", bufs=3) as gxp,
            tc.tile_pool(name="oh", bufs=3) as ohp,
            tc.tile_pool(name="blk", bufs=3) as bp,
            tc.tile_pool(name="dram", bufs=1, space="DRAM") as dram,
        ):
            w1_sb = cp.tile([d_in, d_hid], f32)
            w2_sb = cp.tile([d_hid, d_out], f32)
            idx_sb = cp.tile([P, 8 * totc], mybir.dt.int16)
            dloc_sb = cp.tile([P, totc], bf16)
            iota_b = cp.tile([P, cmax_blk * P], bf16)
            nc.sync.dma_start(out=w1_sb[:], in_=w1_ap[:])
            nc.sync.dma_start(out=w2_sb[:], in_=w2_ap[:])
            nc.sync.dma_start(out=idx_sb[:], in_=idx_ap[:])
            nc.sync.dma_start(out=dloc_sb[:], in_=dloc_ap[:])
            nc.gpsimd.iota(
                iota_b[:].rearrange("p (c m) -> p c m", m=P),
                pattern=[[0, cmax_blk], [1, P]],
                base=0,
                channel_multiplier=0,
                allow_small_or_imprecise_dtypes=True,
            )

            hslice = dram.tile([npc, d_hid], bf16)
            hfull = dram.tile([n_nodes, d_hid], bf16, addr_space="Shared")
            tacc = cp.tile([P, nblk * d_out], f32)  # phase-2 logits, all blocks

            def onehot(col, nch):
                """one-hot [128 edges, nch chunks * 128 nodes], bf16."""
                n = int(nch)
                oh = ohp.tile([P, cmax_blk * P], bf16, tag="oh")
                nc.vector.tensor_tensor(
                    out=oh[:, : n * P].rearrange("p (c m) -> p c m", m=P),
                    in0=dloc_sb[:, col : col + n, None].to_broadcast([P, n, P]),
                    in1=iota_b[:, : n * P].rearrange("p (c m) -> p c m", m=P),
                    op=mybir.AluOpType.is_equal,
                )
                return oh

            def phase(layer):
                """layer 1: gather x_bf16, agg -> @W1f +degb1, relu -> h slice.
                layer 2: gather h_bf16, agg -> @W2f +degb2, log_softmax -> out."""
                table = xb_ap if layer == 1 else hfull[:]
                d_row = d_in if layer == 1 else d_hid
                for g in range(nsb):
                    bs = sb_blocks[g]
                    gt = gxp.tile([P, sbmax * d_row], bf16, tag="g")
                    c0 = col_of[bs[0], "lo"]
                    gather(
                        gt[:, : sb_cl[g] * d_row],
                        table[:SPLIT, :],
                        idx_sb,
                        c0,
                        sb_cl[g],
                        d_row,
                    )
                    gather(
                        gt[:, sb_cl[g] * d_row : (sb_cl[g] + sb_ch[g]) * d_row],
                        table[SPLIT:, :],
                        idx_sb,
                        c0 + sb_cl[g],
                        sb_ch[g],
                        d_row,
                    )
                    for b in bs:
                        nbsz = min(P, npc - b * P)
                        # aggT[f, node] += sum over the block's chunks
                        aggT = (psA if layer == 1 else psA2).tile(
                            [P, P], f32, space="PSUM", tag="aggT"
                        )
                        chunks = [
                            (col_of[b, "lo"], int(CL[b])),
                            (col_of[b, "hi"], int(CH[b])),
                        ]
                        ctot = int(CL[b] + CH[b])
                        kk = 0
                        for cstart, cn in chunks:
                            oh = onehot(cstart, cn)
                            for k in range(cn):
                                gcol = (cstart - c0) * d_row
                                nc.tensor.matmul(
                                    out=aggT[:d_row, :],
                                    lhsT=gt[:, gcol + k * d_row : gcol + (k + 1) * d_row],
                                    rhs=oh[:, k * P : (k + 1) * P],
                                    start=(kk == 0),
                                    stop=(kk == ctot - 1),
                                )
                                kk += 1
                        aggT_sb = bp.tile([P, P], f32, tag="aggT_sb")
                        nc.vector.tensor_copy(out=aggT_sb[:d_row, :], in_=aggT[:d_row, :])
                        d_o = d_hid if layer == 1 else d_out
                        w_sb = w1_sb if layer == 1 else w2_sb
                        degb = degb1_ap if layer == 1 else degb2_ap
                        o_ps = (psH if layer == 1 else psO).tile(
                            [P, d_o], f32, space="PSUM", tag="o"
                        )
                        nc.tensor.matmul(
                            out=o_ps[:], lhsT=aggT_sb[:d_row, :], rhs=w_sb[:],
                            start=True, stop=True,
                        )
                        db = bp.tile([P, d_o], f32, tag="db")
                        nc.sync.dma_start(
                            out=db[:nbsz, :], in_=degb[b * P : b * P + nbsz, :]
                        )
                        if layer == 1:
                            t_sb = bp.tile([P, d_o], f32, tag="t_sb")
                            nc.vector.tensor_add(
                                out=t_sb[:nbsz, :], in0=o_ps[:nbsz, :], in1=db[:nbsz, :]
                            )
                            hb = bp.tile([P, d_hid], bf16, tag="hb")
                            nc.scalar.activation(
                                out=hb[:nbsz, :], in_=t_sb[:nbsz, :],
                                func=mybir.ActivationFunctionType.Relu,
                            )
                            nc.sync.dma_start(
                                out=hslice[b * P : b * P + nbsz, :], in_=hb[:nbsz, :]
                            )
                        else:
                            # accumulate logits; log_softmax batched at the end
                            nc.vector.tensor_add(
                                out=tacc[:nbsz, b * d_o : b * d_o + d_o],
                                in0=o_ps[:nbsz, :],
                                in1=db[:nbsz, :],
                            )
                if layer == 2:
                    # batched log_softmax over all blocks: [P, nblk, d_out]
                    v3 = tacc[:].rearrange("p (b f) -> p b f", f=d_out)
                    nmx = bp.tile([P, nblk], f32, tag="nmx")
                    nc.vector.reduce_max(
                        out=nmx[:], in_=v3, axis=mybir.AxisListType.X, negate=True
                    )
                    tm = cp.tile([P, nblk * d_out], f32)
                    tm3 = tm[:].rearrange("p (b f) -> p b f", f=d_out)
                    nc.vector.tensor_tensor(
                        out=tm3, in0=v3,
                        in1=nmx[:, :, None].to_broadcast([P, nblk, d_out]),
                        op=mybir.AluOpType.add,
                    )
                    exa = cp.tile([P, nblk * d_out], f32)
                    nc.scalar.activation(
                        out=exa[:], in_=tm[:], func=mybir.ActivationFunctionType.Exp
                    )
                    sm = bp.tile([P, nblk], f32, tag="sm")
                    nc.vector.reduce_sum(
                        out=sm[:], in_=exa[:].rearrange("p (b f) -> p b f", f=d_out),
                        axis=mybir.AxisListType.X,
                    )
                    ln = bp.tile([P, nblk], f32, tag="ln")
                    nc.scalar.activation(
                        out=ln[:], in_=sm[:], func=mybir.ActivationFunctionType.Ln
                    )
                    res = exa  # reuse: exp values are consumed
                    nc.vector.tensor_tensor(
                        out=res[:].rearrange("p (b f) -> p b f", f=d_out),
                        in0=tm3,
                        in1=ln[:, :, None].to_broadcast([P, nblk, d_out]),
                        op=mybir.AluOpType.subtract,
                    )
                    for b in range(nblk):
                        nbsz = min(P, npc - b * P)
                        nc.sync.dma_start(
                            out=out_ap[b * P : b * P + nbsz, :],
                            in_=res[:nbsz, b * d_out : (b + 1) * d_out],
                        )

            with (
                tc.tile_pool(name="psA", bufs=2, space="PSUM") as psA,
                tc.tile_pool(name="psH", bufs=2, space="PSUM") as psH,
            ):
                phase(1)

            nc.gpsimd.collective_compute(
                "AllGather",
                mybir.AluOpType.bypass,
                replica_groups=[list(range(n_cores))],
                ins=[hslice[:].opt()],
                outs=[hfull[:].opt()],
            )

            with (
                tc.tile_pool(name="psA2", bufs=2, space="PSUM") as psA2,
                tc.tile_pool(name="psO", bufs=2, space="PSUM") as psO,
            ):
                phase(2)

    nc.compile()
    return nc


_PROGRAM_CACHE = {}


def _run(x, edge_index, W1f, b1f, W2f, b2f, n_cores=N_CORES):
    n_nodes, d_in = x.shape
    d_hid = W1f.shape[1]
    d_out = W2f.shape[1]
    npc = n_nodes // n_cores

    CL, CH, col_of, idx_arrs, dloc_arrs, deg = _preprocess(edge_index, n_nodes, n_cores)

    key = (n_nodes, d_in, d_hid, d_out, n_cores, tuple(CL), tuple(CH))
    if key not in _PROGRAM_CACHE:
        _PROGRAM_CACHE[key] = _build_program(
            CL, CH, col_of, n_nodes, d_in, d_hid, d_out, n_cores
        )
    nc = _PROGRAM_CACHE[key]

    import ml_dtypes

    xb = np.ascontiguousarray(x.astype(ml_dtypes.bfloat16))
    in_maps = []
    for c in range(n_cores):
        deg_c = deg[c * npc : (c + 1) * npc]
        in_maps.append(
            {
                "xb": xb,
                "w1": np.ascontiguousarray(W1f),
                "w2": np.ascontiguousarray(W2f),
                "idx": idx_arrs[c],
                "dloc": np.ascontiguousarray(dloc_arrs[c].astype(ml_dtypes.bfloat16)),
                "degb1": np.ascontiguousarray(np.outer(deg_c, b1f).astype(np.float32)),
                "degb2": np.ascontiguousarray(np.outer(deg_c, b2f).astype(np.float32)),
            }
        )
    res = run_bass_kernel_spmd(
        nc,
        in_maps,
        core_ids=list(range(n_cores)),
        trace=bool(os.environ.get("KERNEL_TRACE")),
    )
    out = np.concatenate([res.results[c]["out"] for c in range(n_cores)], axis=0)
    return out, res


def kernel(x, edge_index, W1, b1, Wa1, ba1, curv1, W2, b2, Wa2, ba2, curv2):
    x = np.asarray(x, dtype=np.float32)
    edge_index = np.asarray(edge_index).astype(np.int64)
    s1 = -abs(float(np.asarray(curv1).reshape(-1)[0]))
    s2 = -abs(float(np.asarray(curv2).reshape(-1)[0]))
    W1f = np.asarray(W1, dtype=np.float32) * s1
    b1f = np.asarray(b1, dtype=np.float32) * s1
    W2f = np.asarray(W2, dtype=np.float32) * s2
    b2f = np.asarray(b2, dtype=np.float32) * s2
    out, _ = _run(x, edge_index, W1f, b1f, W2f, b2f)
    return out


# revision 9
# speedup vs baseline: 1.3300x; 1.2904x over previous
"""HGCN 2-layer GNN message passing kernel for 8 Trainium2 NeuronCores.

Math notes (vs the reference):
  - alpha = softmax over a size-1 axis == 1.0 exactly, so the attention
    branch (Wa, ba, leaky_relu, softmax) contributes nothing.
  - msg = x_j * (-|curv|), so each layer is
        out = segment_sum((x @ W + b)[src], dst) * s      with s = -|curv|
    and since matmul distributes over the segment sum:
        out = segment_sum(x[src], dst) @ (W*s) + deg * (b*s)
    i.e. aggregate raw features first, apply the (scaled) linear after.
  - layer1: h = relu(out1); layer2: log_softmax(out2).

Sharding: nodes are range-partitioned across the 8 cores by destination
(6250 nodes each).  Each core processes the edges whose dst lands in its
range.  Edges are sorted by dst on the host; per 128-node dst block the
core gathers x[src] rows with dma_gather (int16 indices force a low/high
table split at 32768) and segment-sums them with one-hot matmuls
accumulated in PSUM.  Gather tables are bf16 (PSUM accumulation stays
f32).  Between layers the per-core h slices are AllGathered so every core
can gather layer-2 messages from the full table.
"""

import os
import sys

import numpy as np

if "/opt/trn_rl_repo" not in sys.path:
    sys.path.insert(0, "/opt/trn_rl_repo")

import concourse.bacc as bacc
import concourse.bass as bass
import concourse.mybir as mybir
import concourse.tile as tile
from concourse.bass_utils import run_bass_kernel_spmd

P = 128
N_CORES = 8
SPLIT = 32768  # int16 index limit for dma_gather
GBLK = 4  # dst blocks per gather superblock


# ---------------------------------------------------------------------------
# host-side edge preprocessing
# ---------------------------------------------------------------------------

def _wrap_idx(raw):
    """[n*128] row indices -> [128, n*8] int16 dma_gather index layout
    (wrapped in 16 partitions, replicated across the 8 gpsimd cores)."""
    n = raw.shape[0]
    w = raw.reshape(n // 16, 16).T.astype(np.int16)  # [16, n//16]
    return np.tile(w, (8, 1))


def _preprocess(edge_index, n_nodes, n_cores):
    """Sort edges (plus self loops) by dst, split per core / per 128-dst
    block / by src<SPLIT, and build the padded per-core index arrays.

    Chunk column order groups blocks into superblocks of GBLK so each
    superblock needs just two dma_gathers (low table / high table):
      [sb0: lows of b0..b3 | highs of b0..b3][sb1: ...]
    """
    src = np.concatenate([edge_index[0], np.arange(n_nodes, dtype=np.int64)])
    dst = np.concatenate([edge_index[1], np.arange(n_nodes, dtype=np.int64)])
    order = np.argsort(dst, kind="stable")
    src_s = src[order].astype(np.int64)
    dst_s = dst[order].astype(np.int64)
    deg = np.bincount(dst, minlength=n_nodes).astype(np.float32)

    npc = n_nodes // n_cores  # nodes per core
    nblk = (npc + P - 1) // P
    lows, highs, dlows, dhighs = {}, {}, {}, {}
    cl = np.zeros((n_cores, nblk), dtype=np.int64)
    ch = np.zeros((n_cores, nblk), dtype=np.int64)
    for c in range(n_cores):
        for b in range(nblk):
            lo = c * npc + b * P
            hi = min(lo + P, (c + 1) * npc)
            e0 = np.searchsorted(dst_s, lo, side="left")
            e1 = np.searchsorted(dst_s, hi, side="left")
            s = src_s[e0:e1]
            dl = (dst_s[e0:e1] - lo).astype(np.float32)
            m = s < SPLIT
            lows[c, b], dlows[c, b] = s[m], dl[m]
            highs[c, b], dhighs[c, b] = s[~m] - SPLIT, dl[~m]
            cl[c, b] = (len(lows[c, b]) + P - 1) // P
            ch[c, b] = (len(highs[c, b]) + P - 1) // P
    CL = np.maximum(cl.max(axis=0), 1)  # shared program: max chunks per block
    CH = np.maximum(ch.max(axis=0), 1)
    totc = int(CL.sum() + CH.sum())

    # chunk-column order: per superblock, lows of its blocks then highs
    nsb = (nblk + GBLK - 1) // GBLK
    col_of = {}  # (b, "lo"/"hi") -> first chunk column
    col = 0
    for g in range(nsb):
        bs = range(g * GBLK, min((g + 1) * GBLK, nblk))
        for b in bs:
            col_of[b, "lo"] = col
            col += int(CL[b])
        for b in bs:
            col_of[b, "hi"] = col
            col += int(CH[b])
    assert col == totc

    idx_arrs, dloc_arrs = [], []
    for c in range(n_cores):
        idx_np = np.zeros((P, 8 * totc), dtype=np.int16)
        dloc_np = np.full((P, totc), float(P), dtype=np.float32)
        for b in range(nblk):
            for key, arrs, darrs, nch in (
                ("lo", lows, dlows, CL[b]),
                ("hi", highs, dhighs, CH[b]),
            ):
                a = arrs[c, b]
                d = darrs[c, b]
                ni = int(nch) * P
                pad_a = np.zeros(ni, dtype=np.int64)
                pad_a[: len(a)] = a
                pad_d = np.full(ni, float(P), dtype=np.float32)
                pad_d[: len(d)] = d
                c0 = col_of[b, key]
                idx_np[:, 8 * c0 : 8 * (c0 + int(nch))] = _wrap_idx(pad_a)
                dloc_np[:, c0 : c0 + int(nch)] = pad_d.reshape(int(nch), P).T
        idx_arrs.append(idx_np)
        dloc_arrs.append(dloc_np.astype(np.float32))
    return CL, CH, col_of, idx_arrs, dloc_arrs, deg


# ---------------------------------------------------------------------------
# device program
# ---------------------------------------------------------------------------

def _build_program(CL, CH, col_of, n_nodes, d_in, d_hid, d_out, n_cores):
    npc = n_nodes // n_cores
    nblk = len(CL)
    nsb = (nblk + GBLK - 1) // GBLK
    totc = int(CL.sum() + CH.sum())
    f32 = mybir.dt.float32
    bf16 = mybir.dt.bfloat16
    # max chunks handled by one superblock gather tile
    sb_blocks = [list(range(g * GBLK, min((g + 1) * GBLK, nblk))) for g in range(nsb)]
    sb_cl = [int(sum(CL[b] for b in bs)) for bs in sb_blocks]
    sb_ch = [int(sum(CH[b] for b in bs)) for bs in sb_blocks]
    sbmax = max(cl + ch for cl, ch in zip(sb_cl, sb_ch))
    cmax_blk = int((CL + CH).max())

    nc = bacc.Bacc(
        "TRN2",
        target_bir_lowering=False,
        debug=False,
        num_devices=n_cores,
        num_swdge_queues=4,
    )
    xb_ap = nc.dram_tensor("xb", [n_nodes, d_in], bf16, kind="ExternalInput").ap()
    w1_ap = nc.dram_tensor("w1", [d_in, d_hid], f32, kind="ExternalInput").ap()
    w2_ap = nc.dram_tensor("w2", [d_hid, d_out], f32, kind="ExternalInput").ap()
    idx_ap = nc.dram_tensor("idx", [P, 8 * totc], mybir.dt.int16, kind="ExternalInput").ap()
    dloc_ap = nc.dram_tensor("dloc", [P, totc], bf16, kind="ExternalInput").ap()
    degb1_ap = nc.dram_tensor("degb1", [npc, d_hid], f32, kind="ExternalInput").ap()
    degb2_ap = nc.dram_tensor("degb2", [npc, d_out], f32, kind="ExternalInput").ap()
    out_ap = nc.dram_tensor("out", [npc, d_out], f32, kind="ExternalOutput").ap()

    gq = [0]  # rotating swdge queue assignment

    def gather(gt, table, idx_sb, col, nch, elem):
        ni = int(nch) * P
        nc.gpsimd.dma_gather(
            out_ap=gt.rearrange("p (c e) -> p c e", e=elem),
            in_ap=table,
            idxs_ap=idx_sb[:, 8 * col : 8 * (col + int(nch))],
            num_idxs=ni,
            num_idxs_reg=ni,
            elem_size=elem,
            single_packet=False,
            queue_num=gq[0] % 4,
        )
        gq[0] += 1

    with tile.TileContext(nc) as tc:
        with (
            tc.tile_pool(name="const", bufs=1) as cp,
            tc.tile_pool(name="g", bufs=3) as gxp,
            tc.tile_pool(name="oh", bufs=3) as ohp,
            tc.tile_pool(name="blk", bufs=3) as bp,
            tc.tile_pool(name="dram", bufs=1, space="DRAM") as dram,
        ):
            w1_sb = cp.tile([d_in, d_hid], f32)
            w2_sb = cp.tile([d_hid, d_out], f32)
            idx_sb = cp.tile([P, 8 * totc], mybir.dt.int16)
            dloc_sb = cp.tile([P, totc], bf16)
            iota_b = cp.tile([P, cmax_blk * P], bf16)
            nc.sync.dma_start(out=w1_sb[:], in_=w1_ap[:])
            nc.sync.dma_start(out=w2_sb[:], in_=w2_ap[:])
            nc.sync.dma_start(out=idx_sb[:], in_=idx_ap[:])
            nc.sync.dma_start(out=dloc_sb[:], in_=dloc_ap[:])
            nc.gpsimd.iota(
                iota_b[:].rearrange("p (c m) -> p c m", m=P),
                pattern=[[0, cmax_blk], [1, P]],
                base=0,
                channel_multiplier=0,
                allow_small_or_imprecise_dtypes=True,
            )

            hslice = dram.tile([npc, d_hid], bf16)
            hfull = dram.tile([n_nodes, d_hid], bf16, addr_space="Shared")
            hlocal = dram.tile([n_nodes, d_hid], bf16)
            tacc = cp.tile([P, nblk * d_out], f32)  # phase-2 logits, all blocks

            def onehot(col, nch):
                """one-hot [128 edges, nch chunks * 128 nodes], bf16."""
                n = int(nch)
                oh = ohp.tile([P, cmax_blk * P], bf16, tag="oh")
                nc.vector.tensor_tensor(
                    out=oh[:, : n * P].rearrange("p (c m) -> p c m", m=P),
                    in0=dloc_sb[:, col : col + n, None].to_broadcast([P, n, P]),
                    in1=iota_b[:, : n * P].rearrange("p (c m) -> p c m", m=P),
                    op=mybir.AluOpType.is_equal,
                )
                return oh

            def phase(layer):
                """layer 1: gather x_bf16, agg -> @W1f +degb1, relu -> h slice.
                layer 2: gather h_bf16, agg -> @W2f +degb2, log_softmax -> out."""
                table = xb_ap if layer == 1 else hlocal[:]
                d_row = d_in if layer == 1 else d_hid
                for g in range(nsb):
                    bs = sb_blocks[g]
                    gt = gxp.tile([P, sbmax * d_row], bf16, tag="g")
                    c0 = col_of[bs[0], "lo"]
                    gather(
                        gt[:, : sb_cl[g] * d_row],
                        table[:SPLIT, :],
                        idx_sb,
                        c0,
                        sb_cl[g],
                        d_row,
                    )
                    gather(
                        gt[:, sb_cl[g] * d_row : (sb_cl[g] + sb_ch[g]) * d_row],
                        table[SPLIT:, :],
                        idx_sb,
                        c0 + sb_cl[g],
                        sb_ch[g],
                        d_row,
                    )
                    for b in bs:
                        nbsz = min(P, npc - b * P)
                        # aggT[f, node] += sum over the block's chunks
                        aggT = (psA if layer == 1 else psA2).tile(
                            [P, P], f32, space="PSUM", tag="aggT"
                        )
                        chunks = [
                            (col_of[b, "lo"], int(CL[b])),
                            (col_of[b, "hi"], int(CH[b])),
                        ]
                        ctot = int(CL[b] + CH[b])
                        kk = 0
                        for cstart, cn in chunks:
                            oh = onehot(cstart, cn)
                            for k in range(cn):
                                gcol = (cstart - c0) * d_row
                                nc.tensor.matmul(
                                    out=aggT[:d_row, :],
                                    lhsT=gt[:, gcol + k * d_row : gcol + (k + 1) * d_row],
                                    rhs=oh[:, k * P : (k + 1) * P],
                                    start=(kk == 0),
                                    stop=(kk == ctot - 1),
                                )
                                kk += 1
                        aggT_sb = bp.tile([P, P], f32, tag="aggT_sb")
                        nc.vector.tensor_copy(out=aggT_sb[:d_row, :], in_=aggT[:d_row, :])
                        d_o = d_hid if layer == 1 else d_out
                        w_sb = w1_sb if layer == 1 else w2_sb
                        degb = degb1_ap if layer == 1 else degb2_ap
                        o_ps = (psH if layer == 1 else psO).tile(
                            [P, d_o], f32, space="PSUM", tag="o"
                        )
                        nc.tensor.matmul(
                            out=o_ps[:], lhsT=aggT_sb[:d_row, :], rhs=w_sb[:],
                            start=True, stop=True,
                        )
                        db = bp.tile([P, d_o], f32, tag="db")
                        nc.sync.dma_start(
                            out=db[:nbsz, :], in_=degb[b * P : b * P + nbsz, :]
                        )
                        if layer == 1:
                            t_sb = bp.tile([P, d_o], f32, tag="t_sb")
                            nc.vector.tensor_add(
                                out=t_sb[:nbsz, :], in0=o_ps[:nbsz, :], in1=db[:nbsz, :]
                            )
                            hb = bp.tile([P, d_hid], bf16, tag="hb")
                            nc.scalar.activation(
                                out=hb[:nbsz, :], in_=t_sb[:nbsz, :],
                                func=mybir.ActivationFunctionType.Relu,
                            )
                            nc.sync.dma_start(
                                out=hslice[b * P : b * P + nbsz, :], in_=hb[:nbsz, :]
                            )
                        else:
                            # accumulate logits; log_softmax batched at the end
                            nc.vector.tensor_add(
                                out=tacc[:nbsz, b * d_o : b * d_o + d_o],
                                in0=o_ps[:nbsz, :],
                                in1=db[:nbsz, :],
                            )
                if layer == 2:
                    # batched log_softmax over all blocks: [P, nblk, d_out]
                    v3 = tacc[:].rearrange("p (b f) -> p b f", f=d_out)
                    nmx = bp.tile([P, nblk], f32, tag="nmx")
                    nc.vector.reduce_max(
                        out=nmx[:], in_=v3, axis=mybir.AxisListType.X, negate=True
                    )
                    tm = cp.tile([P, nblk * d_out], f32)
                    tm3 = tm[:].rearrange("p (b f) -> p b f", f=d_out)
                    nc.vector.tensor_tensor(
                        out=tm3, in0=v3,
                        in1=nmx[:, :, None].to_broadcast([P, nblk, d_out]),
                        op=mybir.AluOpType.add,
                    )
                    exa = cp.tile([P, nblk * d_out], f32)
                    nc.scalar.activation(
                        out=exa[:], in_=tm[:], func=mybir.ActivationFunctionType.Exp
                    )
                    sm = bp.tile([P, nblk], f32, tag="sm")
                    nc.vector.reduce_sum(
                        out=sm[:], in_=exa[:].rearrange("p (b f) -> p b f", f=d_out),
                        axis=mybir.AxisListType.X,
                    )
                    ln = bp.tile([P, nblk], f32, tag="ln")
                    nc.scalar.activation(
                        out=ln[:], in_=sm[:], func=mybir.ActivationFunctionType.Ln
                    )
                    res = exa  # reuse: exp values are consumed
                    nc.vector.tensor_tensor(
                        out=res[:].rearrange("p (b f) -> p b f", f=d_out),
                        in0=tm3,
                        in1=ln[:, :, None].to_broadcast([P, nblk, d_out]),
                        op=mybir.AluOpType.subtract,
                    )
                    for b in range(nblk):
                        nbsz = min(P, npc - b * P)
                        nc.sync.dma_start(
                            out=out_ap[b * P : b * P + nbsz, :],
                            in_=res[:nbsz, b * d_out : (b + 1) * d_out],
                        )

            with (
                tc.tile_pool(name="psA", bufs=2, space="PSUM") as psA,
                tc.tile_pool(name="psH", bufs=2, space="PSUM") as psH,
            ):
                phase(1)

            nc.gpsimd.collective_compute(
                "AllGather",
                mybir.AluOpType.bypass,
                replica_groups=[list(range(n_cores))],
                ins=[hslice[:].opt()],
                outs=[hfull[:].opt()],
            )

            # bounce the gathered table out of the Shared window: gathers
            # from Shared DRAM are much slower than from regular DRAM
            nc.sync.dma_start(out=hlocal[:], in_=hfull[:])

            with (
                tc.tile_pool(name="psA2", bufs=2, space="PSUM") as psA2,
                tc.tile_pool(name="psO", bufs=2, space="PSUM") as psO,
            ):
                phase(2)

    nc.compile()
    return nc


_PROGRAM_CACHE = {}


def _run(x, edge_index, W1f, b1f, W2f, b2f, n_cores=N_CORES):
    n_nodes, d_in = x.shape
    d_hid = W1f.shape[1]
    d_out = W2f.shape[1]
    npc = n_nodes // n_cores

    CL, CH, col_of, idx_arrs, dloc_arrs, deg = _preprocess(edge_index, n_nodes, n_cores)

    key = (n_nodes, d_in, d_hid, d_out, n_cores, tuple(CL), tuple(CH))
    if key not in _PROGRAM_CACHE:
        _PROGRAM_CACHE[key] = _build_program(
            CL, CH, col_of, n_nodes, d_in, d_hid, d_out, n_cores
        )
    nc = _PROGRAM_CACHE[key]

    import ml_dtypes

    xb = np.ascontiguousarray(x.astype(ml_dtypes.bfloat16))
    in_maps = []
    for c in range(n_cores):
        deg_c = deg[c * npc : (c + 1) * npc]
        in_maps.append(
            {
                "xb": xb,
                "w1": np.ascontiguousarray(W1f),
                "w2": np.ascontiguousarray(W2f),
                "idx": idx_arrs[c],
                "dloc": np.ascontiguousarray(dloc_arrs[c].astype(ml_dtypes.bfloat16)),
                "degb1": np.ascontiguousarray(np.outer(deg_c, b1f).astype(np.float32)),
                "degb2": np.ascontiguousarray(np.outer(deg_c, b2f).astype(np.float32)),
            }
        )
    res = run_bass_kernel_spmd(
        nc,
        in_maps,
        core_ids=list(range(n_cores)),
        trace=bool(os.environ.get("KERNEL_TRACE")),
    )
    out = np.concatenate([res.results[c]["out"] for c in range(n_cores)], axis=0)
    return out, res


def kernel(x, edge_index, W1, b1, Wa1, ba1, curv1, W2, b2, Wa2, ba2, curv2):
    x = np.asarray(x, dtype=np.float32)
    edge_index = np.asarray(edge_index).astype(np.int64)
    s1 = -abs(float(np.asarray(curv1).reshape(-1)[0]))
    s2 = -abs(float(np.asarray(curv2).reshape(-1)[0]))
    W1f = np.asarray(W1, dtype=np.float32) * s1
    b1f = np.asarray(b1, dtype=np.float32) * s1
    W2f = np.asarray(W2, dtype=np.float32) * s2
    b2f = np.asarray(b2, dtype=np.float32) * s2
    out, _ = _run(x, edge_index, W1f, b1f, W2f, b2f)
    return out


# revision 12
# speedup vs baseline: 1.4850x; 1.1166x over previous
"""HGCN 2-layer GNN message passing kernel for 8 Trainium2 NeuronCores.

Math notes (vs the reference):
  - alpha = softmax over a size-1 axis == 1.0 exactly, so the attention
    branch (Wa, ba, leaky_relu, softmax) contributes nothing.
  - msg = x_j * (-|curv|), so each layer is
        out = segment_sum((x @ W + b)[src], dst) * s      with s = -|curv|
    and since matmul distributes over the segment sum:
        out = segment_sum(x[src], dst) @ (W*s) + deg * (b*s)
    i.e. aggregate raw features first, apply the (scaled) linear after.
  - layer1: h = relu(out1); layer2: log_softmax(out2).

Sharding: nodes are range-partitioned across the 8 cores by destination
(6250 nodes each).  Each core processes the edges whose dst lands in its
range.  Edges are sorted by dst on the host; per 128-node dst block the
core gathers x[src] rows with dma_gather (int16 indices force splitting
each gather table in two) and segment-sums them with one-hot matmuls
accumulated in PSUM.  Gather tables are bf16 (PSUM accumulation stays
f32).  The inter-layer exchange of h is two AllGathers: the first fires
mid-phase-1 (its inputs are the first 25 blocks) so most of its latency
overlaps phase-1 compute; phase-2 chunks are grouped by which half-table
their src row lives in so early superblocks only need the first half.
"""

import os
import sys

import numpy as np

if "/opt/trn_rl_repo" not in sys.path:
    sys.path.insert(0, "/opt/trn_rl_repo")

import concourse.bacc as bacc
import concourse.bass as bass
import concourse.mybir as mybir
import concourse.tile as tile
from concourse.bass_utils import run_bass_kernel_spmd

P = 128
N_CORES = 8
SPLIT = 25000  # phase-1 x-table split point (int16 index limit)
GBLK = 4  # dst blocks per gather superblock
ABLK = 25  # blocks per core in the "A" half of the h exchange
LEAD = 3  # phase-2 A-gathers emitted ahead of the first B-gather


def _wrap_idx(raw):
    """[n*128] row indices -> [128, n*8] int16 dma_gather index layout
    (wrapped in 16 partitions, replicated across the 8 gpsimd cores)."""
    n = raw.shape[0]
    w = raw.reshape(n // 16, 16).T.astype(np.int16)  # [16, n//16]
    return np.tile(w, (8, 1))


def _group_edges(srcmaps, n_nodes, n_cores, which):
    """Per (core, dst-block) split edges into two groups and build padded
    per-core index/dst-local arrays with superblock-contiguous columns.

    srcmaps: (src_sorted, dst_sorted) int64 arrays sorted by dst.
    which(src) -> (group 0/1, row id within that group's table).
    Returns CA, CB (chunks per block, maxed over cores), col_of, idx
    arrays, dloc arrays.
    """
    src_s, dst_s = srcmaps
    npc = n_nodes // n_cores
    nblk = (npc + P - 1) // P
    grp = {}
    ca = np.zeros((n_cores, nblk), dtype=np.int64)
    cb = np.zeros((n_cores, nblk), dtype=np.int64)
    g_of, row_of = which(src_s)
    for c in range(n_cores):
        for b in range(nblk):
            lo = c * npc + b * P
            hi = min(lo + P, (c + 1) * npc)
            e0 = np.searchsorted(dst_s, lo, side="left")
            e1 = np.searchsorted(dst_s, hi, side="left")
            g = g_of[e0:e1]
            r = row_of[e0:e1]
            dl = (dst_s[e0:e1] - lo).astype(np.float32)
            grp[c, b, 0] = (r[g == 0], dl[g == 0])
            grp[c, b, 1] = (r[g == 1], dl[g == 1])
            ca[c, b] = (len(grp[c, b, 0][0]) + P - 1) // P
            cb[c, b] = (len(grp[c, b, 1][0]) + P - 1) // P
    CA = np.maximum(ca.max(axis=0), 1)
    CB = np.maximum(cb.max(axis=0), 1)
    totc = int(CA.sum() + CB.sum())

    nsb = (nblk + GBLK - 1) // GBLK
    col_of = {}
    col = 0
    for g in range(nsb):
        bs = range(g * GBLK, min((g + 1) * GBLK, nblk))
        for b in bs:
            col_of[b, 0] = col
            col += int(CA[b])
        for b in bs:
            col_of[b, 1] = col
            col += int(CB[b])
    assert col == totc

    idx_arrs, dloc_arrs = [], []
    for c in range(n_cores):
        idx_np = np.zeros((P, 8 * totc), dtype=np.int16)
        dloc_np = np.full((P, totc), float(P), dtype=np.float32)
        for b in range(nblk):
            for key, nch in ((0, CA[b]), (1, CB[b])):
                a, d = grp[c, b, key]
                ni = int(nch) * P
                pad_a = np.zeros(ni, dtype=np.int64)
                pad_a[: len(a)] = a
                pad_d = np.full(ni, float(P), dtype=np.float32)
                pad_d[: len(d)] = d
                c0 = col_of[b, key]
                idx_np[:, 8 * c0 : 8 * (c0 + int(nch))] = _wrap_idx(pad_a)
                dloc_np[:, c0 : c0 + int(nch)] = pad_d.reshape(int(nch), P).T
        idx_arrs.append(idx_np)
        dloc_arrs.append(dloc_np)
    return CA, CB, col_of, idx_arrs, dloc_arrs


def _preprocess(edge_index, n_nodes, n_cores):
    src = np.concatenate([edge_index[0], np.arange(n_nodes, dtype=np.int64)])
    dst = np.concatenate([edge_index[1], np.arange(n_nodes, dtype=np.int64)])
    order = np.argsort(dst, kind="stable")
    src_s = src[order].astype(np.int64)
    dst_s = dst[order].astype(np.int64)
    deg = np.bincount(dst, minlength=n_nodes).astype(np.float32)
    npc = n_nodes // n_cores
    nblk = (npc + P - 1) // P
    ablk = min(ABLK, (nblk + 1) // 2)
    arows = ablk * P  # rows per core in half A of the h exchange

    def which1(s):
        return (s >= SPLIT).astype(np.int64), np.where(s < SPLIT, s, s - SPLIT)

    def which2(s):
        r = s % npc
        c = s // npc
        in_a = r < arows
        g = (~in_a).astype(np.int64)
        row = np.where(in_a, c * arows + r, c * (npc - arows) + (r - arows))
        return g, row

    p1 = _group_edges((src_s, dst_s), n_nodes, n_cores, which1)
    p2 = _group_edges((src_s, dst_s), n_nodes, n_cores, which2)
    return p1, p2, deg


# ---------------------------------------------------------------------------
# device program
# ---------------------------------------------------------------------------

def _build_program(p1, p2, n_nodes, d_in, d_hid, d_out, n_cores):
    npc = n_nodes // n_cores
    CL1, CH1, col1, _, _ = p1
    CA2, CB2, col2, _, _ = p2
    nblk = len(CL1)
    nsb = (nblk + GBLK - 1) // GBLK
    totc1 = int(CL1.sum() + CH1.sum())
    totc2 = int(CA2.sum() + CB2.sum())
    f32 = mybir.dt.float32
    bf16 = mybir.dt.bfloat16
    sb_blocks = [list(range(g * GBLK, min((g + 1) * GBLK, nblk))) for g in range(nsb)]
    sb1_lo = [int(sum(CL1[b] for b in bs)) for bs in sb_blocks]
    sb1_hi = [int(sum(CH1[b] for b in bs)) for bs in sb_blocks]
    sb2_a = [int(sum(CA2[b] for b in bs)) for bs in sb_blocks]
    sb2_b = [int(sum(CB2[b] for b in bs)) for bs in sb_blocks]
    sb1max = max(a + b for a, b in zip(sb1_lo, sb1_hi))
    sb2amax = max(sb2_a)
    sb2bmax = max(sb2_b)
    ablk = min(ABLK, (nblk + 1) // 2)
    arows = ablk * P
    brows = npc - arows
    cmax_blk = max(int((CL1 + CH1).max()), int((CA2 + CB2).max()))

    nc = bacc.Bacc(
        "TRN2",
        target_bir_lowering=False,
        debug=False,
        num_devices=n_cores,
        num_swdge_queues=4,
    )
    xb_ap = nc.dram_tensor("xb", [n_nodes, d_in], bf16, kind="ExternalInput").ap()
    w1_ap = nc.dram_tensor("w1", [d_in, d_hid], f32, kind="ExternalInput").ap()
    w2_ap = nc.dram_tensor("w2", [d_hid, d_out], f32, kind="ExternalInput").ap()
    idx1_ap = nc.dram_tensor("idx1", [P, 8 * totc1], mybir.dt.int16, kind="ExternalInput").ap()
    dloc1_ap = nc.dram_tensor("dloc1", [P, totc1], bf16, kind="ExternalInput").ap()
    idx2_ap = nc.dram_tensor("idx2", [P, 8 * totc2], mybir.dt.int16, kind="ExternalInput").ap()
    dloc2_ap = nc.dram_tensor("dloc2", [P, totc2], bf16, kind="ExternalInput").ap()
    degb1_ap = nc.dram_tensor("degb1", [npc, d_hid], f32, kind="ExternalInput").ap()
    degb2_ap = nc.dram_tensor("degb2", [npc, d_out], f32, kind="ExternalInput").ap()
    out_ap = nc.dram_tensor("out", [npc, d_out], f32, kind="ExternalOutput").ap()

    gq = [0]

    def gather(gt, table, idx_sb, col, nch, elem):
        ni = int(nch) * P
        nc.gpsimd.dma_gather(
            out_ap=gt.rearrange("p (c e) -> p c e", e=elem),
            in_ap=table,
            idxs_ap=idx_sb[:, 8 * col : 8 * (col + int(nch))],
            num_idxs=ni,
            num_idxs_reg=ni,
            elem_size=elem,
            single_packet=False,
            queue_num=gq[0] % 4,
        )
        gq[0] += 1

    with tile.TileContext(nc) as tc:
        with (
            tc.tile_pool(name="const", bufs=1) as cp,
            tc.tile_pool(name="oh", bufs=2) as ohp,
            tc.tile_pool(name="blk", bufs=3) as bp,
            tc.tile_pool(name="dram", bufs=1, space="DRAM") as dram,
        ):
            w1_sb = cp.tile([d_in, d_hid], f32)
            w2_sb = cp.tile([d_hid, d_out], f32)
            idx1_sb = cp.tile([P, 8 * totc1], mybir.dt.int16)
            dloc1_sb = cp.tile([P, totc1], bf16)
            idx2_sb = cp.tile([P, 8 * totc2], mybir.dt.int16)
            dloc2_sb = cp.tile([P, totc2], bf16)
            iota_b = cp.tile([P, P], bf16)
            tacc = cp.tile([P, nblk * d_out], f32)  # phase-2 logits
            tm = cp.tile([P, nblk * d_out], f32)
            nc.sync.dma_start(out=w1_sb[:], in_=w1_ap[:])
            nc.sync.dma_start(out=w2_sb[:], in_=w2_ap[:])
            nc.sync.dma_start(out=idx1_sb[:], in_=idx1_ap[:])
            nc.sync.dma_start(out=dloc1_sb[:], in_=dloc1_ap[:])
            nc.sync.dma_start(out=idx2_sb[:], in_=idx2_ap[:])
            nc.sync.dma_start(out=dloc2_sb[:], in_=dloc2_ap[:])
            nc.gpsimd.iota(
                iota_b[:],
                pattern=[[1, P]],
                base=0,
                channel_multiplier=0,
                allow_small_or_imprecise_dtypes=True,
            )

            hsliceA = dram.tile([arows, d_hid], bf16)
            hsliceB = dram.tile([brows, d_hid], bf16)
            hfullA = dram.tile([n_cores * arows, d_hid], bf16)
            hfullB = dram.tile([n_cores * brows, d_hid], bf16)

            def onehot(dloc_sb, col, nch):
                """one-hot [128 edges, nch chunks * 128 nodes], bf16."""
                n = int(nch)
                oh = ohp.tile([P, cmax_blk * P], bf16, tag="oh")
                nc.vector.tensor_tensor(
                    out=oh[:, : n * P].rearrange("p (c m) -> p c m", m=P),
                    in0=dloc_sb[:, col : col + n, None].to_broadcast([P, n, P]),
                    in1=iota_b[:, None, :].to_broadcast([P, n, P]),
                    op=mybir.AluOpType.is_equal,
                )
                return oh

            def block_compute(layer, b, gt, gcol0, dloc_sb, colmap, CA, CB, psA, psO):
                """Segment-sum the block's chunks from gt, then the linear
                and layer epilogue."""
                nbsz = min(P, npc - b * P)
                d_row = d_in if layer == 1 else d_hid
                aggT = psA.tile([P, P], f32, space="PSUM", tag="aggT")
                ctot = int(CA[b] + CB[b])
                kk = 0
                for key, cn in ((0, int(CA[b])), (1, int(CB[b]))):
                    cstart = colmap[b, key]
                    oh = onehot(dloc_sb, cstart, cn)
                    for k in range(cn):
                        gcol = (cstart - gcol0[key]) * d_row
                        nc.tensor.matmul(
                            out=aggT[:d_row, :],
                            lhsT=gt[key][:, gcol + k * d_row : gcol + (k + 1) * d_row],
                            rhs=oh[:, k * P : (k + 1) * P],
                            start=(kk == 0),
                            stop=(kk == ctot - 1),
                        )
                        kk += 1
                aggT_sb = bp.tile([P, P], f32, tag="aggT_sb")
                nc.vector.tensor_copy(out=aggT_sb[:d_row, :], in_=aggT[:d_row, :])
                d_o = d_hid if layer == 1 else d_out
                w_sb = w1_sb if layer == 1 else w2_sb
                degb = degb1_ap if layer == 1 else degb2_ap
                o_ps = psO.tile([P, d_o], f32, space="PSUM", tag="o")
                nc.tensor.matmul(
                    out=o_ps[:], lhsT=aggT_sb[:d_row, :], rhs=w_sb[:],
                    start=True, stop=True,
                )
                db = bp.tile([P, d_o], f32, tag="db")
                nc.sync.dma_start(out=db[:nbsz, :], in_=degb[b * P : b * P + nbsz, :])
                if layer == 1:
                    t_sb = bp.tile([P, d_o], f32, tag="t_sb")
                    nc.vector.tensor_add(
                        out=t_sb[:nbsz, :], in0=o_ps[:nbsz, :], in1=db[:nbsz, :]
                    )
                    hb = bp.tile([P, d_hid], bf16, tag="hb")
                    nc.scalar.activation(
                        out=hb[:nbsz, :], in_=t_sb[:nbsz, :],
                        func=mybir.ActivationFunctionType.Relu,
                    )
                    if b < ablk:
                        nc.sync.dma_start(
                            out=hsliceA[b * P : b * P + nbsz, :], in_=hb[:nbsz, :]
                        )
                    else:
                        r0 = b * P - arows
                        nc.sync.dma_start(
                            out=hsliceB[r0 : r0 + nbsz, :], in_=hb[:nbsz, :]
                        )
                else:
                    nc.vector.tensor_add(
                        out=tacc[:nbsz, b * d_o : b * d_o + d_o],
                        in0=o_ps[:nbsz, :],
                        in1=db[:nbsz, :],
                    )

            ag_a_done = [False]

            # ---------------- phase 1 (+ AllGather A mid-way) ----------------
            with (
                tc.tile_pool(name="g1", bufs=3) as g1p,
                tc.tile_pool(name="psA", bufs=2, space="PSUM") as psA,
                tc.tile_pool(name="psH", bufs=2, space="PSUM") as psH,
            ):
                for g in range(nsb):
                    bs = sb_blocks[g]
                    gt = g1p.tile([P, sb1max * d_in], bf16, tag="g1")
                    c0 = col1[bs[0], 0]
                    gather(gt[:, : sb1_lo[g] * d_in], xb_ap[:SPLIT, :], idx1_sb, c0, sb1_lo[g], d_in)
                    gather(
                        gt[:, sb1_lo[g] * d_in : (sb1_lo[g] + sb1_hi[g]) * d_in],
                        xb_ap[SPLIT:, :],
                        idx1_sb,
                        c0 + sb1_lo[g],
                        sb1_hi[g],
                        d_in,
                    )
                    for b in bs:
                        block_compute(
                            1, b, {0: gt, 1: gt}, {0: c0, 1: c0}, dloc1_sb, col1,
                            CL1, CH1, psA, psH,
                        )
                    if bs[-1] >= ablk - 1 and not ag_a_done[0]:  # blocks 0..ablk-1 written
                        nc.gpsimd.collective_compute(
                            "AllGather",
                            mybir.AluOpType.bypass,
                            replica_groups=[list(range(n_cores))],
                            ins=[hsliceA[:].opt()],
                            outs=[hfullA[:].opt()],
                        )
                        ag_a_done[0] = True

            nc.gpsimd.collective_compute(
                "AllGather",
                mybir.AluOpType.bypass,
                replica_groups=[list(range(n_cores))],
                ins=[hsliceB[:].opt()],
                outs=[hfullB[:].opt()],
            )

            # ---------------- phase 2 (A-gathers lead B by LEAD sbs) ----------------
            with (
                tc.tile_pool(name="g2a", bufs=LEAD + 2) as g2ap,
                tc.tile_pool(name="g2b", bufs=3) as g2bp,
                tc.tile_pool(name="psA2", bufs=2, space="PSUM") as psA2,
                tc.tile_pool(name="psO", bufs=2, space="PSUM") as psO,
            ):
                gtA = {}
                gtB = {}

                def emit_gA(g):
                    gtA[g] = g2ap.tile([P, sb2amax * d_hid], bf16, tag="g2a", name=f"gta{g}")
                    gather(
                        gtA[g][:, : sb2_a[g] * d_hid], hfullA[:], idx2_sb,
                        col2[sb_blocks[g][0], 0], sb2_a[g], d_hid,
                    )

                def emit_gB(g):
                    gtB[g] = g2bp.tile([P, sb2bmax * d_hid], bf16, tag="g2b", name=f"gtb{g}")
                    gather(
                        gtB[g][:, : sb2_b[g] * d_hid], hfullB[:], idx2_sb,
                        col2[sb_blocks[g][0], 1], sb2_b[g], d_hid,
                    )

                for g in range(min(LEAD, nsb)):
                    emit_gA(g)
                for g in range(nsb):
                    if g + LEAD < nsb:
                        emit_gA(g + LEAD)
                    emit_gB(g)
                    bs = sb_blocks[g]
                    for b in bs:
                        block_compute(
                            2, b, {0: gtA[g], 1: gtB[g]},
                            {0: col2[bs[0], 0], 1: col2[bs[0], 1]},
                            dloc2_sb, col2, CA2, CB2, psA2, psO,
                        )

                # batched log_softmax over all blocks: [P, nblk, d_out]
                v3 = tacc[:].rearrange("p (b f) -> p b f", f=d_out)
                nmx = bp.tile([P, nblk], f32, tag="nmx")
                nc.vector.reduce_max(
                    out=nmx[:], in_=v3, axis=mybir.AxisListType.X, negate=True
                )
                tm3 = tm[:].rearrange("p (b f) -> p b f", f=d_out)
                nc.vector.tensor_tensor(
                    out=tm3, in0=v3,
                    in1=nmx[:, :, None].to_broadcast([P, nblk, d_out]),
                    op=mybir.AluOpType.add,
                )
                nc.scalar.activation(
                    out=tacc[:], in_=tm[:], func=mybir.ActivationFunctionType.Exp
                )
                sm = bp.tile([P, nblk], f32, tag="sm")
                nc.vector.reduce_sum(
                    out=sm[:], in_=tacc[:].rearrange("p (b f) -> p b f", f=d_out),
                    axis=mybir.AxisListType.X,
                )
                ln = bp.tile([P, nblk], f32, tag="ln")
                nc.scalar.activation(
                    out=ln[:], in_=sm[:], func=mybir.ActivationFunctionType.Ln
                )
                nc.vector.tensor_tensor(
                    out=tm3,
                    in0=tm3,
                    in1=ln[:, :, None].to_broadcast([P, nblk, d_out]),
                    op=mybir.AluOpType.subtract,
                )
                for b in range(nblk):
                    nbsz = min(P, npc - b * P)
                    nc.sync.dma_start(
                        out=out_ap[b * P : b * P + nbsz, :],
                        in_=tm[:nbsz, b * d_out : (b + 1) * d_out],
                    )

    nc.compile()
    return nc


_PROGRAM_CACHE = {}


def _run(x, edge_index, W1f, b1f, W2f, b2f, n_cores=N_CORES):
    n_nodes, d_in = x.shape
    d_hid = W1f.shape[1]
    d_out = W2f.shape[1]
    npc = n_nodes // n_cores

    p1, p2, deg = _preprocess(edge_index, n_nodes, n_cores)
    CL1, CH1, col1, idx1_arrs, dloc1_arrs = p1
    CA2, CB2, col2, idx2_arrs, dloc2_arrs = p2

    key = (n_nodes, d_in, d_hid, d_out, n_cores, tuple(CL1), tuple(CH1), tuple(CA2), tuple(CB2))
    if key not in _PROGRAM_CACHE:
        _PROGRAM_CACHE[key] = _build_program(p1, p2, n_nodes, d_in, d_hid, d_out, n_cores)
    nc = _PROGRAM_CACHE[key]

    import ml_dtypes

    xb = np.ascontiguousarray(x.astype(ml_dtypes.bfloat16))
    in_maps = []
    for c in range(n_cores):
        deg_c = deg[c * npc : (c + 1) * npc]
        in_maps.append(
            {
                "xb": xb,
                "w1": np.ascontiguousarray(W1f),
                "w2": np.ascontiguousarray(W2f),
                "idx1": idx1_arrs[c],
                "dloc1": np.ascontiguousarray(dloc1_arrs[c].astype(ml_dtypes.bfloat16)),
                "idx2": idx2_arrs[c],
                "dloc2": np.ascontiguousarray(dloc2_arrs[c].astype(ml_dtypes.bfloat16)),
                "degb1": np.ascontiguousarray(np.outer(deg_c, b1f).astype(np.float32)),
                "degb2": np.ascontiguousarray(np.outer(deg_c, b2f).astype(np.float32)),
            }
        )
    res = run_bass_kernel_spmd(
        nc,
        in_maps,
        core_ids=list(range(n_cores)),
        trace=bool(os.environ.get("KERNEL_TRACE")),
    )
    out = np.concatenate([res.results[c]["out"] for c in range(n_cores)], axis=0)
    return out, res


def kernel(x, edge_index, W1, b1, Wa1, ba1, curv1, W2, b2, Wa2, ba2, curv2):
    x = np.asarray(x, dtype=np.float32)
    edge_index = np.asarray(edge_index).astype(np.int64)
    s1 = -abs(float(np.asarray(curv1).reshape(-1)[0]))
    s2 = -abs(float(np.asarray(curv2).reshape(-1)[0]))
    W1f = np.asarray(W1, dtype=np.float32) * s1
    b1f = np.asarray(b1, dtype=np.float32) * s1
    W2f = np.asarray(W2, dtype=np.float32) * s2
    b2f = np.asarray(b2, dtype=np.float32) * s2
    out, _ = _run(x, edge_index, W1f, b1f, W2f, b2f)
    return out


# revision 13
# speedup vs baseline: 1.6815x; 1.1323x over previous
"""HGCN 2-layer GNN message passing kernel for 8 Trainium2 NeuronCores.

Math notes (vs the reference):
  - alpha = softmax over a size-1 axis == 1.0 exactly, so the attention
    branch (Wa, ba, leaky_relu, softmax) contributes nothing.
  - msg = x_j * (-|curv|), so each layer is
        out = segment_sum((x @ W + b)[src], dst) * s      with s = -|curv|
    and since matmul distributes over the segment sum:
        out = segment_sum(x[src], dst) @ (W*s) + deg * (b*s)
    i.e. aggregate raw features first, apply the (scaled) linear after.
  - layer1: h = relu(out1); layer2: log_softmax(out2).

Sharding: nodes are range-partitioned across the 8 cores by destination
(6250 nodes each).  Each core processes the edges whose dst lands in its
range.  Edges are sorted by dst on the host; per 128-node dst block the
core gathers x[src] rows with dma_gather (int16 indices force splitting
each gather table in two) and segment-sums them with one-hot matmuls
accumulated in PSUM.  Gather tables are bf16 (PSUM accumulation stays
f32).  The inter-layer exchange of h is two AllGathers: the first fires
mid-phase-1 (its inputs are the first 25 blocks) so most of its latency
overlaps phase-1 compute; phase-2 chunks are grouped by which half-table
their src row lives in so early superblocks only need the first half.
"""

import os
import sys

import numpy as np

if "/opt/trn_rl_repo" not in sys.path:
    sys.path.insert(0, "/opt/trn_rl_repo")

import concourse.bacc as bacc
import concourse.bass as bass
import concourse.mybir as mybir
import concourse.tile as tile
from concourse.bass_utils import run_bass_kernel_spmd

P = 128
N_CORES = 8
SPLIT = 25000  # phase-1 x-table split point (int16 index limit)
GBLK = 2  # dst blocks per gather superblock
ABLK = 25  # blocks per core in the "A" half of the h exchange
LEAD = 6  # phase-2 A-gathers emitted ahead of the first B-gather


def _wrap_idx(raw):
    """[n*128] row indices -> [128, n*8] int16 dma_gather index layout
    (wrapped in 16 partitions, replicated across the 8 gpsimd cores)."""
    n = raw.shape[0]
    w = raw.reshape(n // 16, 16).T.astype(np.int16)  # [16, n//16]
    return np.tile(w, (8, 1))


def _group_edges(srcmaps, n_nodes, n_cores, which):
    """Per (core, dst-block) split edges into two groups and build padded
    per-core index/dst-local arrays with superblock-contiguous columns.

    srcmaps: (src_sorted, dst_sorted) int64 arrays sorted by dst.
    which(src) -> (group 0/1, row id within that group's table).
    Returns CA, CB (chunks per block, maxed over cores), col_of, idx
    arrays, dloc arrays.
    """
    src_s, dst_s = srcmaps
    npc = n_nodes // n_cores
    nblk = (npc + P - 1) // P
    grp = {}
    ca = np.zeros((n_cores, nblk), dtype=np.int64)
    cb = np.zeros((n_cores, nblk), dtype=np.int64)
    g_of, row_of = which(src_s)
    for c in range(n_cores):
        for b in range(nblk):
            lo = c * npc + b * P
            hi = min(lo + P, (c + 1) * npc)
            e0 = np.searchsorted(dst_s, lo, side="left")
            e1 = np.searchsorted(dst_s, hi, side="left")
            g = g_of[e0:e1]
            r = row_of[e0:e1]
            dl = (dst_s[e0:e1] - lo).astype(np.float32)
            grp[c, b, 0] = (r[g == 0], dl[g == 0])
            grp[c, b, 1] = (r[g == 1], dl[g == 1])
            ca[c, b] = (len(grp[c, b, 0][0]) + P - 1) // P
            cb[c, b] = (len(grp[c, b, 1][0]) + P - 1) // P
    CA = np.maximum(ca.max(axis=0), 1)
    CB = np.maximum(cb.max(axis=0), 1)
    totc = int(CA.sum() + CB.sum())

    nsb = (nblk + GBLK - 1) // GBLK
    col_of = {}
    col = 0
    for g in range(nsb):
        bs = range(g * GBLK, min((g + 1) * GBLK, nblk))
        for b in bs:
            col_of[b, 0] = col
            col += int(CA[b])
        for b in bs:
            col_of[b, 1] = col
            col += int(CB[b])
    assert col == totc

    idx_arrs, dloc_arrs = [], []
    for c in range(n_cores):
        idx_np = np.zeros((P, 8 * totc), dtype=np.int16)
        dloc_np = np.full((P, totc), float(P), dtype=np.float32)
        for b in range(nblk):
            for key, nch in ((0, CA[b]), (1, CB[b])):
                a, d = grp[c, b, key]
                ni = int(nch) * P
                pad_a = np.zeros(ni, dtype=np.int64)
                pad_a[: len(a)] = a
                pad_d = np.full(ni, float(P), dtype=np.float32)
                pad_d[: len(d)] = d
                c0 = col_of[b, key]
                idx_np[:, 8 * c0 : 8 * (c0 + int(nch))] = _wrap_idx(pad_a)
                dloc_np[:, c0 : c0 + int(nch)] = pad_d.reshape(int(nch), P).T
        idx_arrs.append(idx_np)
        dloc_arrs.append(dloc_np)
    return CA, CB, col_of, idx_arrs, dloc_arrs


def _preprocess(edge_index, n_nodes, n_cores):
    src = np.concatenate([edge_index[0], np.arange(n_nodes, dtype=np.int64)])
    dst = np.concatenate([edge_index[1], np.arange(n_nodes, dtype=np.int64)])
    order = np.argsort(dst, kind="stable")
    src_s = src[order].astype(np.int64)
    dst_s = dst[order].astype(np.int64)
    deg = np.bincount(dst, minlength=n_nodes).astype(np.float32)
    npc = n_nodes // n_cores
    nblk = (npc + P - 1) // P
    ablk = min(ABLK, (nblk + 1) // 2)
    arows = ablk * P  # rows per core in half A of the h exchange

    def which1(s):
        return (s >= SPLIT).astype(np.int64), np.where(s < SPLIT, s, s - SPLIT)

    def which2(s):
        r = s % npc
        c = s // npc
        in_a = r < arows
        g = (~in_a).astype(np.int64)
        row = np.where(in_a, c * arows + r, c * (npc - arows) + (r - arows))
        return g, row

    p1 = _group_edges((src_s, dst_s), n_nodes, n_cores, which1)
    p2 = _group_edges((src_s, dst_s), n_nodes, n_cores, which2)
    return p1, p2, deg


# ---------------------------------------------------------------------------
# device program
# ---------------------------------------------------------------------------

def _build_program(p1, p2, n_nodes, d_in, d_hid, d_out, n_cores):
    npc = n_nodes // n_cores
    CL1, CH1, col1, _, _ = p1
    CA2, CB2, col2, _, _ = p2
    nblk = len(CL1)
    nsb = (nblk + GBLK - 1) // GBLK
    totc1 = int(CL1.sum() + CH1.sum())
    totc2 = int(CA2.sum() + CB2.sum())
    f32 = mybir.dt.float32
    bf16 = mybir.dt.bfloat16
    sb_blocks = [list(range(g * GBLK, min((g + 1) * GBLK, nblk))) for g in range(nsb)]
    sb1_lo = [int(sum(CL1[b] for b in bs)) for bs in sb_blocks]
    sb1_hi = [int(sum(CH1[b] for b in bs)) for bs in sb_blocks]
    sb2_a = [int(sum(CA2[b] for b in bs)) for bs in sb_blocks]
    sb2_b = [int(sum(CB2[b] for b in bs)) for bs in sb_blocks]
    sb1max = max(a + b for a, b in zip(sb1_lo, sb1_hi))
    sb2amax = max(sb2_a)
    sb2bmax = max(sb2_b)
    ablk = min(ABLK, (nblk + 1) // 2)
    arows = ablk * P
    brows = npc - arows
    cmax_blk = max(int((CL1 + CH1).max()), int((CA2 + CB2).max()))

    nc = bacc.Bacc(
        "TRN2",
        target_bir_lowering=False,
        debug=False,
        num_devices=n_cores,
        num_swdge_queues=4,
    )
    xb_ap = nc.dram_tensor("xb", [n_nodes, d_in], bf16, kind="ExternalInput").ap()
    w1_ap = nc.dram_tensor("w1", [d_in, d_hid], f32, kind="ExternalInput").ap()
    w2_ap = nc.dram_tensor("w2", [d_hid, d_out], f32, kind="ExternalInput").ap()
    idx1_ap = nc.dram_tensor("idx1", [P, 8 * totc1], mybir.dt.int16, kind="ExternalInput").ap()
    dloc1_ap = nc.dram_tensor("dloc1", [P, totc1], bf16, kind="ExternalInput").ap()
    idx2_ap = nc.dram_tensor("idx2", [P, 8 * totc2], mybir.dt.int16, kind="ExternalInput").ap()
    dloc2_ap = nc.dram_tensor("dloc2", [P, totc2], bf16, kind="ExternalInput").ap()
    degb1_ap = nc.dram_tensor("degb1", [npc, d_hid], f32, kind="ExternalInput").ap()
    degb2_ap = nc.dram_tensor("degb2", [npc, d_out], f32, kind="ExternalInput").ap()
    out_ap = nc.dram_tensor("out", [npc, d_out], f32, kind="ExternalOutput").ap()

    gq = [0]

    def gather(gt, table, idx_sb, col, nch, elem):
        ni = int(nch) * P
        nc.gpsimd.dma_gather(
            out_ap=gt.rearrange("p (c e) -> p c e", e=elem),
            in_ap=table,
            idxs_ap=idx_sb[:, 8 * col : 8 * (col + int(nch))],
            num_idxs=ni,
            num_idxs_reg=ni,
            elem_size=elem,
            single_packet=False,
            queue_num=gq[0] % 4,
        )
        gq[0] += 1

    with tile.TileContext(nc) as tc:
        with (
            tc.tile_pool(name="const", bufs=1) as cp,
            tc.tile_pool(name="oh", bufs=2) as ohp,
            tc.tile_pool(name="blk", bufs=3) as bp,
            tc.tile_pool(name="dram", bufs=1, space="DRAM") as dram,
        ):
            w1_sb = cp.tile([d_in, d_hid], f32)
            w2_sb = cp.tile([d_hid, d_out], f32)
            idx1_sb = cp.tile([P, 8 * totc1], mybir.dt.int16)
            dloc1_sb = cp.tile([P, totc1], bf16)
            idx2_sb = cp.tile([P, 8 * totc2], mybir.dt.int16)
            dloc2_sb = cp.tile([P, totc2], bf16)
            iota_b = cp.tile([P, P], bf16)
            tacc = cp.tile([P, nblk * d_out], f32)  # phase-2 logits
            tm = cp.tile([P, nblk * d_out], f32)
            nc.sync.dma_start(out=w1_sb[:], in_=w1_ap[:])
            nc.sync.dma_start(out=w2_sb[:], in_=w2_ap[:])
            nc.sync.dma_start(out=idx1_sb[:], in_=idx1_ap[:])
            nc.sync.dma_start(out=dloc1_sb[:], in_=dloc1_ap[:])
            nc.sync.dma_start(out=idx2_sb[:], in_=idx2_ap[:])
            nc.sync.dma_start(out=dloc2_sb[:], in_=dloc2_ap[:])
            nc.gpsimd.iota(
                iota_b[:],
                pattern=[[1, P]],
                base=0,
                channel_multiplier=0,
                allow_small_or_imprecise_dtypes=True,
            )

            hsliceA = dram.tile([arows, d_hid], bf16)
            hsliceB = dram.tile([brows, d_hid], bf16)
            hfullA = dram.tile([n_cores * arows, d_hid], bf16)
            hfullB = dram.tile([n_cores * brows, d_hid], bf16)

            def onehot(dloc_sb, col, nch):
                """one-hot [128 edges, nch chunks * 128 nodes], bf16."""
                n = int(nch)
                oh = ohp.tile([P, cmax_blk * P], bf16, tag="oh")
                nc.vector.tensor_tensor(
                    out=oh[:, : n * P].rearrange("p (c m) -> p c m", m=P),
                    in0=dloc_sb[:, col : col + n, None].to_broadcast([P, n, P]),
                    in1=iota_b[:, None, :].to_broadcast([P, n, P]),
                    op=mybir.AluOpType.is_equal,
                )
                return oh

            def block_compute(layer, b, gt, gcol0, dloc_sb, colmap, CA, CB, psA, psO):
                """Segment-sum the block's chunks from gt, then the linear
                and layer epilogue."""
                nbsz = min(P, npc - b * P)
                d_row = d_in if layer == 1 else d_hid
                aggT = psA.tile([P, P], f32, space="PSUM", tag="aggT")
                ctot = int(CA[b] + CB[b])
                kk = 0
                for key, cn in ((0, int(CA[b])), (1, int(CB[b]))):
                    cstart = colmap[b, key]
                    oh = onehot(dloc_sb, cstart, cn)
                    for k in range(cn):
                        gcol = (cstart - gcol0[key]) * d_row
                        nc.tensor.matmul(
                            out=aggT[:d_row, :],
                            lhsT=gt[key][:, gcol + k * d_row : gcol + (k + 1) * d_row],
                            rhs=oh[:, k * P : (k + 1) * P],
                            start=(kk == 0),
                            stop=(kk == ctot - 1),
                        )
                        kk += 1
                aggT_sb = bp.tile([P, P], f32, tag="aggT_sb")
                nc.vector.tensor_copy(out=aggT_sb[:d_row, :], in_=aggT[:d_row, :])
                d_o = d_hid if layer == 1 else d_out
                w_sb = w1_sb if layer == 1 else w2_sb
                degb = degb1_ap if layer == 1 else degb2_ap
                o_ps = psO.tile([P, d_o], f32, space="PSUM", tag="o")
                nc.tensor.matmul(
                    out=o_ps[:], lhsT=aggT_sb[:d_row, :], rhs=w_sb[:],
                    start=True, stop=True,
                )
                db = bp.tile([P, d_o], f32, tag="db")
                nc.sync.dma_start(out=db[:nbsz, :], in_=degb[b * P : b * P + nbsz, :])
                if layer == 1:
                    t_sb = bp.tile([P, d_o], f32, tag="t_sb")
                    nc.vector.tensor_add(
                        out=t_sb[:nbsz, :], in0=o_ps[:nbsz, :], in1=db[:nbsz, :]
                    )
                    hb = bp.tile([P, d_hid], bf16, tag="hb")
                    nc.scalar.activation(
                        out=hb[:nbsz, :], in_=t_sb[:nbsz, :],
                        func=mybir.ActivationFunctionType.Relu,
                    )
                    if b < ablk:
                        nc.sync.dma_start(
                            out=hsliceA[b * P : b * P + nbsz, :], in_=hb[:nbsz, :]
                        )
                    else:
                        r0 = b * P - arows
                        nc.sync.dma_start(
                            out=hsliceB[r0 : r0 + nbsz, :], in_=hb[:nbsz, :]
                        )
                else:
                    nc.vector.tensor_add(
                        out=tacc[:nbsz, b * d_o : b * d_o + d_o],
                        in0=o_ps[:nbsz, :],
                        in1=db[:nbsz, :],
                    )

            ag_a_done = [False]

            # ---------------- phase 1 (+ AllGather A mid-way) ----------------
            with (
                tc.tile_pool(name="g1lo", bufs=8) as g1lop,
                tc.tile_pool(name="g1hi", bufs=8) as g1hip,
                tc.tile_pool(name="psA", bufs=2, space="PSUM") as psA,
                tc.tile_pool(name="psH", bufs=2, space="PSUM") as psH,
            ):
                sb1lomax = max(sb1_lo)
                sb1himax = max(sb1_hi)
                for g in range(nsb):
                    bs = sb_blocks[g]
                    c0 = col1[bs[0], 0]
                    gtlo = g1lop.tile([P, sb1lomax * d_in], bf16, tag="g1lo")
                    gather(gtlo[:, : sb1_lo[g] * d_in], xb_ap[:SPLIT, :], idx1_sb, c0, sb1_lo[g], d_in)
                    gthi = g1hip.tile([P, sb1himax * d_in], bf16, tag="g1hi")
                    gather(
                        gthi[:, : sb1_hi[g] * d_in],
                        xb_ap[SPLIT:, :],
                        idx1_sb,
                        c0 + sb1_lo[g],
                        sb1_hi[g],
                        d_in,
                    )
                    for b in bs:
                        block_compute(
                            1, b, {0: gtlo, 1: gthi},
                            {0: c0, 1: c0 + sb1_lo[g]}, dloc1_sb, col1,
                            CL1, CH1, psA, psH,
                        )
                    if bs[-1] >= ablk - 1 and not ag_a_done[0]:  # blocks 0..ablk-1 written
                        nc.gpsimd.collective_compute(
                            "AllGather",
                            mybir.AluOpType.bypass,
                            replica_groups=[list(range(n_cores))],
                            ins=[hsliceA[:].opt()],
                            outs=[hfullA[:].opt()],
                        )
                        ag_a_done[0] = True

            nc.gpsimd.collective_compute(
                "AllGather",
                mybir.AluOpType.bypass,
                replica_groups=[list(range(n_cores))],
                ins=[hsliceB[:].opt()],
                outs=[hfullB[:].opt()],
            )

            # ---------------- phase 2 (A-gathers lead B by LEAD sbs) ----------------
            with (
                tc.tile_pool(name="g2a", bufs=LEAD + 2) as g2ap,
                tc.tile_pool(name="g2b", bufs=8) as g2bp,
                tc.tile_pool(name="psA2", bufs=2, space="PSUM") as psA2,
                tc.tile_pool(name="psO", bufs=2, space="PSUM") as psO,
            ):
                gtA = {}
                gtB = {}

                def emit_gA(g):
                    gtA[g] = g2ap.tile([P, sb2amax * d_hid], bf16, tag="g2a", name=f"gta{g}")
                    gather(
                        gtA[g][:, : sb2_a[g] * d_hid], hfullA[:], idx2_sb,
                        col2[sb_blocks[g][0], 0], sb2_a[g], d_hid,
                    )

                def emit_gB(g):
                    gtB[g] = g2bp.tile([P, sb2bmax * d_hid], bf16, tag="g2b", name=f"gtb{g}")
                    gather(
                        gtB[g][:, : sb2_b[g] * d_hid], hfullB[:], idx2_sb,
                        col2[sb_blocks[g][0], 1], sb2_b[g], d_hid,
                    )

                for g in range(min(LEAD, nsb)):
                    emit_gA(g)
                for g in range(nsb):
                    if g + LEAD < nsb:
                        emit_gA(g + LEAD)
                    emit_gB(g)
                    bs = sb_blocks[g]
                    for b in bs:
                        block_compute(
                            2, b, {0: gtA[g], 1: gtB[g]},
                            {0: col2[bs[0], 0], 1: col2[bs[0], 1]},
                            dloc2_sb, col2, CA2, CB2, psA2, psO,
                        )

                # batched log_softmax over all blocks: [P, nblk, d_out]
                v3 = tacc[:].rearrange("p (b f) -> p b f", f=d_out)
                nmx = bp.tile([P, nblk], f32, tag="nmx")
                nc.vector.reduce_max(
                    out=nmx[:], in_=v3, axis=mybir.AxisListType.X, negate=True
                )
                tm3 = tm[:].rearrange("p (b f) -> p b f", f=d_out)
                nc.vector.tensor_tensor(
                    out=tm3, in0=v3,
                    in1=nmx[:, :, None].to_broadcast([P, nblk, d_out]),
                    op=mybir.AluOpType.add,
                )
                nc.scalar.activation(
                    out=tacc[:], in_=tm[:], func=mybir.ActivationFunctionType.Exp
                )
                sm = bp.tile([P, nblk], f32, tag="sm")
                nc.vector.reduce_sum(
                    out=sm[:], in_=tacc[:].rearrange("p (b f) -> p b f", f=d_out),
                    axis=mybir.AxisListType.X,
                )
                ln = bp.tile([P, nblk], f32, tag="ln")
                nc.scalar.activation(
                    out=ln[:], in_=sm[:], func=mybir.ActivationFunctionType.Ln
                )
                nc.vector.tensor_tensor(
                    out=tm3,
                    in0=tm3,
                    in1=ln[:, :, None].to_broadcast([P, nblk, d_out]),
                    op=mybir.AluOpType.subtract,
                )
                for b in range(nblk):
                    nbsz = min(P, npc - b * P)
                    nc.sync.dma_start(
                        out=out_ap[b * P : b * P + nbsz, :],
                        in_=tm[:nbsz, b * d_out : (b + 1) * d_out],
                    )

    nc.compile()
    return nc


_PROGRAM_CACHE = {}


def _run(x, edge_index, W1f, b1f, W2f, b2f, n_cores=N_CORES):
    n_nodes, d_in = x.shape
    d_hid = W1f.shape[1]
    d_out = W2f.shape[1]
    npc = n_nodes // n_cores

    p1, p2, deg = _preprocess(edge_index, n_nodes, n_cores)
    CL1, CH1, col1, idx1_arrs, dloc1_arrs = p1
    CA2, CB2, col2, idx2_arrs, dloc2_arrs = p2

    key = (n_nodes, d_in, d_hid, d_out, n_cores, tuple(CL1), tuple(CH1), tuple(CA2), tuple(CB2))
    if key not in _PROGRAM_CACHE:
        _PROGRAM_CACHE[key] = _build_program(p1, p2, n_nodes, d_in, d_hid, d_out, n_cores)
    nc = _PROGRAM_CACHE[key]

    import ml_dtypes

    xb = np.ascontiguousarray(x.astype(ml_dtypes.bfloat16))
    in_maps = []
    for c in range(n_cores):
        deg_c = deg[c * npc : (c + 1) * npc]
        in_maps.append(
            {
                "xb": xb,
                "w1": np.ascontiguousarray(W1f),
                "w2": np.ascontiguousarray(W2f),
                "idx1": idx1_arrs[c],
                "dloc1": np.ascontiguousarray(dloc1_arrs[c].astype(ml_dtypes.bfloat16)),
                "idx2": idx2_arrs[c],
                "dloc2": np.ascontiguousarray(dloc2_arrs[c].astype(ml_dtypes.bfloat16)),
                "degb1": np.ascontiguousarray(np.outer(deg_c, b1f).astype(np.float32)),
                "degb2": np.ascontiguousarray(np.outer(deg_c, b2f).astype(np.float32)),
            }
        )
    res = run_bass_kernel_spmd(
        nc,
        in_maps,
        core_ids=list(range(n_cores)),
        trace=bool(os.environ.get("KERNEL_TRACE")),
    )
    out = np.concatenate([res.results[c]["out"] for c in range(n_cores)], axis=0)
    return out, res


def kernel(x, edge_index, W1, b1, Wa1, ba1, curv1, W2, b2, Wa2, ba2, curv2):
    x = np.asarray(x, dtype=np.float32)
    edge_index = np.asarray(edge_index).astype(np.int64)
    s1 = -abs(float(np.asarray(curv1).reshape(-1)[0]))
    s2 = -abs(float(np.asarray(curv2).reshape(-1)[0]))
    W1f = np.asarray(W1, dtype=np.float32) * s1
    b1f = np.asarray(b1, dtype=np.float32) * s1
    W2f = np.asarray(W2, dtype=np.float32) * s2
    b2f = np.asarray(b2, dtype=np.float32) * s2
    out, _ = _run(x, edge_index, W1f, b1f, W2f, b2f)
    return out
